# revision 1
# baseline (speedup 1.0000x reference)
"""Trainium2 Bass kernel for GroupedQueryAttention.

Sharding: 8 cores; core c owns KV head g=c and Q heads 4c..4c+3, both batch
elements. Each core computes its [2, 2048, 256] output slice; host concats.

Per-core dataflow (per batch b):
  A) hs [2048, 2048] is loaded row-natural and PE-transposed (is_transpose
     matmul vs identity) into hsT chunks [128 d, 512 s]; projections
     Q^T (2x128 rows), [K^T|V^T] (128 rows) accumulate over the 16 d-tiles.
     1/sqrt(HD) is folded into Wq/bq on the host.
  B) V^T rows are PE-transposed back to natural V [s_k, 64] and a ones
     column is appended -> [V|1] so the PV matmul also produces the softmax
     denominator (row 64 of the output).
  C) Scores are computed transposed, S^T [s_k, s_q]; exp on ACT directly
     PSUM->SBUF (no max subtraction: |scores| < ~6 at this data scale);
     ctxU^T [65, s_q] = [V|1]^T @ expS^T accumulates over s_k tiles in PSUM.
     Small PE transposes bring ctxU^T back to natural [s_q, 65]; DVE does
     1/denominator and the scale-multiply into the output tile.

All matmul operands use float32r (fp32 bits, fast PE path).
"""

import sys
from contextlib import ExitStack

import numpy as np

sys.path.insert(0, "/opt/trn_rl_repo")

import concourse.bass as bass  # noqa: E402
import concourse.bacc as bacc  # noqa: E402
import concourse.tile as tile  # noqa: E402
from concourse import mybir  # noqa: E402
from concourse.bass_utils import run_bass_kernel_spmd  # noqa: E402

B = 2
S = 2048
D = 2048
HD = 64
NCORES = 8
QH = 4           # q heads per core
MCOLS = QH * HD  # 256 output cols per core

MM_DT = mybir.dt.float32r
F32 = mybir.dt.float32
Exp = mybir.ActivationFunctionType.Exp

NDT = 16         # d tiles of 128
NSC = 4          # s chunks of 512 per batch
NKT = 16         # s_k tiles of 128
NSQ = 2          # s_q chunks of 1024


def build_nc():
    nc = bacc.Bacc("TRN2", target_bir_lowering=False, debug=False)

    hs_d = nc.dram_tensor("hs", [B, S, D], MM_DT, kind="ExternalInput")
    wq_d = nc.dram_tensor("wq", [D, MCOLS], MM_DT, kind="ExternalInput")
    wkv_d = nc.dram_tensor("wkv", [D, 128], MM_DT, kind="ExternalInput")
    bq_d = nc.dram_tensor("bq", [128, 2], F32, kind="ExternalInput")
    bkv_d = nc.dram_tensor("bkv", [128, 1], F32, kind="ExternalInput")
    id_d = nc.dram_tensor("ident", [128, 128], MM_DT, kind="ExternalInput")
    out_d = nc.dram_tensor("out", [B, S, MCOLS], F32, kind="ExternalOutput")

    with tile.TileContext(nc) as tc, ExitStack() as ctx:
        const = ctx.enter_context(tc.tile_pool(name="const", bufs=1))
        wqp = ctx.enter_context(tc.tile_pool(name="wqp", bufs=NDT))
        wkvp = ctx.enter_context(tc.tile_pool(name="wkvp", bufs=NDT))
        natp = ctx.enter_context(tc.tile_pool(name="natp", bufs=2))
        hstp = ctx.enter_context(tc.tile_pool(name="hstp", bufs=NDT + 2))
        qtp = ctx.enter_context(tc.tile_pool(name="qtp", bufs=4))
        kvp = ctx.enter_context(tc.tile_pool(name="kvp", bufs=2))
        kthp = ctx.enter_context(tc.tile_pool(name="kthp", bufs=2))
        v1p = ctx.enter_context(tc.tile_pool(name="v1p", bufs=2 * NKT))
        expp = ctx.enter_context(tc.tile_pool(name="expp", bufs=3))
        up = ctx.enter_context(tc.tile_pool(name="up", bufs=2))
        recp = ctx.enter_context(tc.tile_pool(name="recp", bufs=4))
        outp = ctx.enter_context(tc.tile_pool(name="outp", bufs=16))
        psp = ctx.enter_context(tc.tile_pool(name="psp", bufs=3, space="PSUM"))
        pvp = ctx.enter_context(tc.tile_pool(name="pvp", bufs=1, space="PSUM"))

        ident = const.tile([128, 128], MM_DT, tag="ident")
        nc.sync.dma_start(out=ident[:], in_=id_d[:])
        bq_sb = const.tile([128, 2], F32, tag="bq")
        nc.sync.dma_start(out=bq_sb[:], in_=bq_d[:])
        bkv_sb = const.tile([128, 1], F32, tag="bkv")
        nc.sync.dma_start(out=bkv_sb[:], in_=bkv_d[:])
        zb = const.tile([128, 1], F32, tag="zb")
        nc.vector.memset(zb[:], 0.0)
        ones_sb = const.tile([128, 1], F32, tag="ones")
        nc.vector.memset(ones_sb[:], 1.0)
        zero64 = const.tile([128, 64], F32, tag="zero64")
        nc.vector.memset(zero64[:], 0.0)

        wq_sb = []
        wkv_sb = []
        for dt_ in range(NDT):
            w = wqp.tile([128, MCOLS], MM_DT, tag="wq")
            nc.sync.dma_start(out=w[:], in_=wq_d[dt_ * 128:(dt_ + 1) * 128, :])
            wq_sb.append(w)
            w2 = wkvp.tile([128, 128], MM_DT, tag="wkv")
            nc.sync.dma_start(out=w2[:], in_=wkv_d[dt_ * 128:(dt_ + 1) * 128, :])
            wkv_sb.append(w2)

        for b in range(B):
            # ---- Phase A: transpose hs + projections ----
            qT = [qtp.tile([128, S], MM_DT, tag="qt", name=f"qT{b}_{i}") for i in range(2)]
            kvT = kvp.tile([128, S], MM_DT, tag="kv")
            for sc in range(NSC):
                hsT = [hstp.tile([128, 512], MM_DT, tag="hst", name=f"hsT{b}_{sc}_{i}") for i in range(NDT)]
                for st in range(4):
                    r0 = sc * 512 + st * 128
                    nat = natp.tile([128, D], MM_DT, tag="nat")
                    nc.sync.dma_start(out=nat[:], in_=hs_d[b, r0:r0 + 128, :])
                    for dt_ in range(NDT):
                        pst = psp.tile([128, 128], MM_DT, tag="ps")
                        nc.tensor.transpose(
                            pst[:], nat[:, dt_ * 128:(dt_ + 1) * 128], ident[:]
                        )
                        nc.vector.tensor_copy(
                            hsT[dt_][:, st * 128:(st + 1) * 128], pst[:]
                        )
                c0, c1 = sc * 512, (sc + 1) * 512
                for qc in range(2):
                    ps = psp.tile([128, 512], F32, tag="ps")
                    for dt_ in range(NDT):
                        nc.tensor.matmul(
                            ps[:], wq_sb[dt_][:, qc * 128:(qc + 1) * 128],
                            hsT[dt_][:], start=(dt_ == 0), stop=(dt_ == NDT - 1),
                        )
                    nc.vector.tensor_scalar_add(
                        qT[qc][:, c0:c1], ps[:], bq_sb[:, qc:qc + 1]
                    )
                ps = psp.tile([128, 512], F32, tag="ps")
                for dt_ in range(NDT):
                    nc.tensor.matmul(
                        ps[:], wkv_sb[dt_][:], hsT[dt_][:],
                        start=(dt_ == 0), stop=(dt_ == NDT - 1),
                    )
                nc.vector.tensor_scalar_add(kvT[:, c0:c1], ps[:], bkv_sb[:])

            kth = kthp.tile([128, S], MM_DT, tag="kth")
            nc.sync.dma_start(out=kth[64:128, :], in_=kvT[0:64, :])

            # ---- Phase B: V natural + ones column ----
            v1 = []
            for kt in range(NKT):
                pst = psp.tile([128, 64], MM_DT, tag="ps")
                nc.tensor.transpose(
                    pst[:], kvT[64:128, kt * 128:(kt + 1) * 128],
                    ident[64:128, 64:128],
                )
                v = v1p.tile([128, 128], MM_DT, tag="v1")
                nc.vector.tensor_copy(v[:, 0:64], pst[:])
                nc.vector.tensor_copy(v[:, 64:128], zero64[:])
                nc.vector.tensor_copy(v[:, 64:65], ones_sb[:])
                v1.append(v)

            # ---- Phase C: attention ----
            outt = [outp.tile([128, MCOLS], F32, tag="out", name=f"outt{b}_{i}") for i in range(16)]
            for h in range(QH):
                qrow = (h % 2) * 64
                qt = qT[h // 2]
                for sq in range(NSQ):
                    q0 = sq * 1024
                    pv = pvp.tile([128, 1024], F32, tag="pv")
                    for kt in range(NKT):
                        pss = psp.tile([128, 1024], F32, tag="ps")
                        kmat = kvT if qrow == 0 else kth
                        for qc in range(2):
                            nc.tensor.matmul(
                                pss[:, qc * 512:(qc + 1) * 512],
                                kmat[qrow:qrow + 64, kt * 128:(kt + 1) * 128],
                                qt[qrow:qrow + 64,
                                   q0 + qc * 512:q0 + (qc + 1) * 512],
                                start=True, stop=True,
                            )
                        ex = expp.tile([128, 1024], MM_DT, tag="exp")
                        nc.scalar.activation(ex[:], pss[:], Exp, bias=zb[:])
                        for qc in range(2):
                            nc.tensor.matmul(
                                pv[:, qc * 512:(qc + 1) * 512],
                                v1[kt][:], ex[:, qc * 512:(qc + 1) * 512],
                                start=(kt == 0), stop=(kt == NKT - 1),
                            )
                    u = up.tile([128, 1024], MM_DT, tag="u")
                    nc.vector.tensor_copy(u[:], pv[:])
                    for tb in range(8):
                        pst = psp.tile([128, 128], MM_DT, tag="ps")
                        nc.tensor.transpose(
                            pst[:], u[:, tb * 128:(tb + 1) * 128],
                            ident[:],
                        )
                        rec = recp.tile([128, 1], F32, tag="rec")
                        nc.vector.reciprocal(rec[:], pst[:, 64:65])
                        st_i = sq * 8 + tb
                        nc.vector.tensor_scalar_mul(
                            outt[st_i][:, h * 64:(h + 1) * 64],
                            pst[:, 0:64], rec[:],
                        )
            for st_i in range(16):
                nc.sync.dma_start(
                    out=out_d[b, st_i * 128:(st_i + 1) * 128, :],
                    in_=outt[st_i][:],
                )

    nc.compile()
    return nc


def make_in_maps(hidden_states, Wq, bq, Wk, bk, Wv, bv):
    hs = np.ascontiguousarray(np.asarray(hidden_states, dtype=np.float32))
    Wq = np.asarray(Wq, dtype=np.float32)
    bq = np.asarray(bq, dtype=np.float32)
    Wk = np.asarray(Wk, dtype=np.float32)
    bk = np.asarray(bk, dtype=np.float32)
    Wv = np.asarray(Wv, dtype=np.float32)
    bv = np.asarray(bv, dtype=np.float32)
    sc = 1.0 / np.sqrt(np.float32(HD))
    ident = np.eye(128, dtype=np.float32)
    in_maps = []
    for c in range(NCORES):
        qs = slice(c * MCOLS, (c + 1) * MCOLS)
        ks = slice(c * HD, (c + 1) * HD)
        bq_c = (bq[qs] * sc).reshape(2, 128).T
        in_maps.append({
            "hs": hs,
            "wq": np.ascontiguousarray(Wq[:, qs] * sc),
            "wkv": np.ascontiguousarray(
                np.concatenate([Wk[:, ks], Wv[:, ks]], axis=1)),
            "bq": np.ascontiguousarray(bq_c),
            "bkv": np.concatenate([bk[ks], bv[ks]]).reshape(128, 1),
            "ident": ident,
        })
    return in_maps


_NC_CACHE = {}


def get_nc():
    if "nc" not in _NC_CACHE:
        _NC_CACHE["nc"] = build_nc()
    return _NC_CACHE["nc"]


def kernel(hidden_states, Wq, bq, Wk, bk, Wv, bv):
    nc = get_nc()
    in_maps = make_in_maps(hidden_states, Wq, bq, Wk, bk, Wv, bv)
    res = run_bass_kernel_spmd(nc, in_maps, list(range(NCORES)))
    outs = [np.asarray(r["out"], dtype=np.float32) for r in res.results]
    return np.concatenate(outs, axis=-1)



# revision 5
# speedup vs baseline: 1.7435x; 1.7435x over previous
"""Trainium2 Bass kernel for GroupedQueryAttention (v2, bf16).

Sharding: 8 cores; core c owns KV head g=c and Q heads 4c..4c+3, both batch
elements. Each core computes its [2, 2048, 256] output slice; host concats.

Key design vs v1:
  * hs is pre-transposed AND pre-cast to bf16 on the host (hsr layout:
    [b, s-chunk, d-in-tile(128), d-tile(16) x s(512)]), removing all 512
    PE transposes and their DVE drains from the device program.
  * All PE operands are bf16 (1.0 cycles/row at any ap size), PSUM fp32.
  * PV uses expS^T tiles as the STATIONARY and natural [V|1] as the moving
    ([128 k, 65]): output is ctx in natural [q, d] orientation with the
    softmax denominator in column 64 -> no output transposes, and PV costs
    65 rows per (k-tile, q-tile) instead of 512 per (k-tile, 512q).
  * exp runs on ACT in [128, 1024] chunks (double-buffered PSUM), which is
    the phase-C bottleneck; batch 1's projection work is interleaved into
    batch 0's attention loop to fill the PE slack under ACT.
"""

import sys
from contextlib import ExitStack

import numpy as np

sys.path.insert(0, "/opt/trn_rl_repo")

import concourse.bass as bass  # noqa: E402
import concourse.bacc as bacc  # noqa: E402
import concourse.tile as tile  # noqa: E402
from concourse import mybir  # noqa: E402
from concourse.bass_utils import run_bass_kernel_spmd  # noqa: E402

B = 2
S = 2048
D = 2048
HD = 64
NCORES = 8
QH = 4           # q heads per core
MCOLS = QH * HD  # 256 output cols per core

BF = mybir.dt.bfloat16
F32 = mybir.dt.float32
Exp = mybir.ActivationFunctionType.Exp

NDT = 16         # d tiles of 128
NSC = 4          # s chunks of 512 per batch
NKT = 16         # s_k tiles of 128
NQC = 2          # q chunks of 1024 per batch
QTPC = 8         # q tiles of 128 per q chunk


def build_nc():
    nc = bacc.Bacc("TRN2", target_bir_lowering=False, debug=False)

    hsr_d = nc.dram_tensor("hsr", [B, NSC, 128, NDT * 512], BF,
                           kind="ExternalInput")
    wqr_d = nc.dram_tensor("wqr", [128, NDT * 256], BF, kind="ExternalInput")
    wkvr_d = nc.dram_tensor("wkvr", [128, NDT * 128], BF, kind="ExternalInput")
    bq_d = nc.dram_tensor("bq", [128, 2], F32, kind="ExternalInput")
    bkv_d = nc.dram_tensor("bkv", [128, 1], F32, kind="ExternalInput")
    id_d = nc.dram_tensor("ident", [128, 128], BF, kind="ExternalInput")
    out_d = nc.dram_tensor("out", [B, S, MCOLS], F32, kind="ExternalOutput")

    with tile.TileContext(nc) as tc, ExitStack() as ctx:
        const = ctx.enter_context(tc.tile_pool(name="const", bufs=1))
        wqp = ctx.enter_context(tc.tile_pool(name="wqp", bufs=1))
        hstp = ctx.enter_context(tc.tile_pool(name="hstp", bufs=3))
        qtp = ctx.enter_context(tc.tile_pool(name="qtp", bufs=4))
        kvp = ctx.enter_context(tc.tile_pool(name="kvp", bufs=2))
        kthp = ctx.enter_context(tc.tile_pool(name="kthp", bufs=2))
        v1p = ctx.enter_context(tc.tile_pool(name="v1p", bufs=2 * NKT))
        expp = ctx.enter_context(tc.tile_pool(name="expp", bufs=33))
        recp = ctx.enter_context(tc.tile_pool(name="recp", bufs=4))
        outp = ctx.enter_context(tc.tile_pool(name="outp", bufs=16))
        psap = ctx.enter_context(tc.tile_pool(name="psap", bufs=2, space="PSUM"))
        pssp = ctx.enter_context(tc.tile_pool(name="pssp", bufs=2, space="PSUM"))
        ctxp = ctx.enter_context(tc.tile_pool(name="ctxp", bufs=1, space="PSUM"))

        ident = const.tile([128, 128], BF, tag="ident")
        nc.sync.dma_start(out=ident[:], in_=id_d[:])
        bq_sb = const.tile([128, 2], F32, tag="bq")
        nc.sync.dma_start(out=bq_sb[:], in_=bq_d[:])
        bkv_sb = const.tile([128, 1], F32, tag="bkv")
        nc.sync.dma_start(out=bkv_sb[:], in_=bkv_d[:])
        zb = const.tile([128, 1], F32, tag="zb")
        nc.vector.memset(zb[:], 0.0)

        wq_sb = wqp.tile([128, NDT * 256], BF, tag="wq")
        nc.sync.dma_start(out=wq_sb[:], in_=wqr_d[:])
        wkv_sb = wqp.tile([128, NDT * 128], BF, tag="wkv")
        nc.sync.dma_start(out=wkv_sb[:], in_=wkvr_d[:])

        # hsT chunk loads for both batches, issued upfront (slot reuse
        # dependencies serialize them against phase-A consumption).
        hst = [[None] * NSC for _ in range(B)]
        for b in range(B):
            for sc in range(NSC):
                h = hstp.tile([128, NDT * 512], BF, tag="hst",
                              name=f"hst{b}_{sc}")
                nc.sync.dma_start(out=h[:], in_=hsr_d[b, sc])
                hst[b][sc] = h

        qT = [[None, None] for _ in range(B)]
        kvT = [None] * B
        kth = [None] * B
        v1 = [[None] * NKT for _ in range(B)]

        def gen_a(b):
            """Phase A+B for batch b, one engine instruction per yield."""
            qT[b][0] = qtp.tile([128, S], BF, tag="qt", name=f"qT{b}_0")
            qT[b][1] = qtp.tile([128, S], BF, tag="qt", name=f"qT{b}_1")
            kvT[b] = kvp.tile([128, S], BF, tag="kv", name=f"kvT{b}")
            for sc in range(NSC):
                hs_t = hst[b][sc]
                c0 = sc * 512
                for qc in range(2):
                    ps = psap.tile([128, 512], F32, tag="ps")
                    for dt_ in range(NDT):
                        nc.tensor.matmul(
                            ps[:],
                            wq_sb[:, dt_ * 256 + qc * 128:
                                  dt_ * 256 + (qc + 1) * 128],
                            hs_t[:, dt_ * 512:(dt_ + 1) * 512],
                            start=(dt_ == 0), stop=(dt_ == NDT - 1),
                        )
                        yield
                    nc.vector.tensor_scalar_add(
                        qT[b][qc][:, c0:c0 + 512], ps[:], bq_sb[:, qc:qc + 1])
                    yield
                ps = psap.tile([128, 512], F32, tag="ps")
                for dt_ in range(NDT):
                    nc.tensor.matmul(
                        ps[:], wkv_sb[:, dt_ * 128:(dt_ + 1) * 128],
                        hs_t[:, dt_ * 512:(dt_ + 1) * 512],
                        start=(dt_ == 0), stop=(dt_ == NDT - 1),
                    )
                    yield
                nc.vector.tensor_scalar_add(
                    kvT[b][:, c0:c0 + 512], ps[:], bkv_sb[:])
                yield
            # K^T copy shifted to partitions 64:127 for odd heads.
            # Issued from the Pool engine queue so it does not block the
            # SP queue carrying the batch-1 hsT loads.
            kth[b] = kthp.tile([128, S], BF, tag="kth", name=f"kth{b}")
            nc.gpsimd.dma_start(out=kth[b][64:128, :], in_=kvT[b][0:64, :])
            yield
            # V natural tiles [s_k 128, 64] plus a ones column.
            for kt in range(NKT):
                pst = psap.tile([128, 512], BF, tag="ps", name=f"pst{b}_{kt}")
                nc.tensor.transpose(
                    pst[:, 0:64], kvT[b][64:128, kt * 128:(kt + 1) * 128],
                    ident[64:128, 64:128],
                )
                yield
                v = v1p.tile([128, 65], BF, tag="v1", name=f"v1_{b}_{kt}")
                nc.vector.memset(v[:, 64:65], 1.0)
                yield
                nc.vector.tensor_copy(v[:, 0:64], pst[:, 0:64])
                yield
                v1[b][kt] = v

        def emit_c(b, interleave):
            """Phase C for batch b.

            PV for chunk (qc, h) is software-pipelined into the scores/exp
            loop of the NEXT chunk (8 PV matmuls per kt iteration), so the
            ctx accumulation groups in each PSUM zero region are strictly
            sequential per q-tile.  `interleave` (batch b+1's phase A) is
            advanced ~2 engine instructions per kt iteration to fill the PE
            slack under the ACT-bound exp stream.
            """
            outt = [None] * (NQC * QTPC)
            exs = {}
            ctxs = {}

            def coff(qi):
                # qi 0..6 packed in bank 0; qi 7 at the bank-1 boundary so no
                # accumulation group straddles a PSUM bank.
                return qi * 65 if qi < 7 else 512

            def pv_chunk(key, it):
                qc, h = key
                ctx_t = ctxs[key]
                qi = it // 2
                base = (it % 2) * 8
                for k2 in range(8):
                    kt = base + k2
                    nc.tensor.matmul(
                        ctx_t[:, coff(qi):coff(qi) + 65],
                        exs[key][kt][:, qi * 128:(qi + 1) * 128],
                        v1[b][kt][:],
                        start=(it % 2 == 0 and k2 == 0),
                        stop=(it % 2 == 1 and k2 == 7),
                    )

            def normalize_qi(key, qi):
                qc, h = key
                ctx_t = ctxs[key]
                qtile = qc * QTPC + qi
                if h == 0:
                    outt[qtile] = outp.tile([128, MCOLS], F32, tag="out",
                                            name=f"outt{b}_{qtile}")
                rec = recp.tile([128, 1], F32, tag="rec")
                nc.vector.reciprocal(
                    rec[:], ctx_t[:, coff(qi) + 64:coff(qi) + 65])
                nc.vector.tensor_scalar_mul(
                    outt[qtile][:, h * 64:(h + 1) * 64],
                    ctx_t[:, coff(qi):coff(qi) + 64], rec[:])
                if h == QH - 1:
                    nc.sync.dma_start(
                        out=out_d[b, qtile * 128:(qtile + 1) * 128, :],
                        in_=outt[qtile][:])

            order = [(qc, h) for qc in range(NQC) for h in range(QH)]
            prev = None
            for key in order:
                qc, h = key
                r0 = (h % 2) * 64
                kmat = kvT[b] if r0 == 0 else kth[b]
                qt = qT[b][h // 2]
                ctxs[key] = ctxp.tile([128, 577], F32, tag="ctx",
                                      name=f"ctx{b}_{qc}_{h}")
                exs[key] = []
                for kt in range(NKT):
                    pss = pssp.tile([128, 1024], F32, tag="pss")
                    for j in range(2):
                        q0 = qc * 1024 + j * 512
                        nc.tensor.matmul(
                            pss[:, j * 512:(j + 1) * 512],
                            kmat[r0:r0 + 64, kt * 128:(kt + 1) * 128],
                            qt[r0:r0 + 64, q0:q0 + 512],
                            start=True, stop=True,
                        )
                    ex = expp.tile([128, 1024], BF, tag="ex")
                    nc.scalar.activation(ex[:], pss[:], Exp, bias=zb[:])
                    exs[key].append(ex)
                    if prev is not None:
                        pv_chunk(prev, kt)
                        if kt % 2 == 1:
                            normalize_qi(prev, kt // 2)
                    if interleave is not None:
                        for _ in range(2):
                            if next(interleave, StopIteration) is StopIteration:
                                interleave = None
                                break
                if prev is not None:
                    del exs[prev]
                    del ctxs[prev]
                prev = key
            for it in range(NKT):
                pv_chunk(prev, it)
                if it % 2 == 1:
                    normalize_qi(prev, it // 2)
            return interleave

        for _ in gen_a(0):
            pass
        g1 = gen_a(1)
        g1 = emit_c(0, g1)
        if g1 is not None:
            for _ in g1:
                pass
        emit_c(1, None)

    nc.compile()
    return nc


def make_in_maps(hidden_states, Wq, bq, Wk, bk, Wv, bv):
    bf = mybir.dt.np(BF)
    hs = np.asarray(hidden_states, dtype=np.float32)
    Wq = np.asarray(Wq, dtype=np.float32)
    bq = np.asarray(bq, dtype=np.float32)
    Wk = np.asarray(Wk, dtype=np.float32)
    bk = np.asarray(bk, dtype=np.float32)
    Wv = np.asarray(Wv, dtype=np.float32)
    bv = np.asarray(bv, dtype=np.float32)
    sc = 1.0 / np.sqrt(np.float32(HD))
    # [b, sc, p(d in tile), t(d tile), j(s in chunk)] -> [2, 4, 128, 8192]
    hsr = np.ascontiguousarray(
        hs.reshape(B, NSC, 512, NDT, 128).transpose(0, 1, 4, 3, 2)
        .reshape(B, NSC, 128, NDT * 512).astype(bf))
    ident = np.eye(128, dtype=bf)
    in_maps = []
    for c in range(NCORES):
        qs = slice(c * MCOLS, (c + 1) * MCOLS)
        ks = slice(c * HD, (c + 1) * HD)
        wqs = (Wq[:, qs] * sc).astype(bf)
        wqr = np.ascontiguousarray(
            wqs.reshape(NDT, 128, MCOLS).transpose(1, 0, 2)
            .reshape(128, NDT * MCOLS))
        wkvs = np.concatenate([Wk[:, ks], Wv[:, ks]], axis=1).astype(bf)
        wkvr = np.ascontiguousarray(
            wkvs.reshape(NDT, 128, 128).transpose(1, 0, 2)
            .reshape(128, NDT * 128))
        bq_c = np.ascontiguousarray((bq[qs] * sc).reshape(2, 128).T)
        in_maps.append({
            "hsr": hsr,
            "wqr": wqr,
            "wkvr": wkvr,
            "bq": bq_c,
            "bkv": np.concatenate([bk[ks], bv[ks]]).reshape(128, 1),
            "ident": ident,
        })
    return in_maps


_NC_CACHE = {}


def get_nc():
    if "nc" not in _NC_CACHE:
        _NC_CACHE["nc"] = build_nc()
    return _NC_CACHE["nc"]


def kernel(hidden_states, Wq, bq, Wk, bk, Wv, bv):
    nc = get_nc()
    in_maps = make_in_maps(hidden_states, Wq, bq, Wk, bk, Wv, bv)
    res = run_bass_kernel_spmd(nc, in_maps, list(range(NCORES)))
    outs = [np.asarray(r["out"], dtype=np.float32) for r in res.results]
    return np.concatenate(outs, axis=-1)


# revision 9
# speedup vs baseline: 1.7699x; 1.0151x over previous
"""Trainium2 Bass kernel for GroupedQueryAttention (v3, bf16, early-start).

Sharding: 8 cores; core c owns KV head g=c and Q heads 4c..4c+3, both batch
elements. Each core computes its [2, 2048, 256] output slice; host concats.

Design:
  * hs is pre-transposed AND pre-cast to bf16 on the host (hsr layout:
    [b, s-chunk, d-in-tile(128), d-tile(16) x s(512)]), removing all 512
    PE transposes and their DVE drains from the device program.
  * All PE operands are bf16 (1.0 cycles/row at any ap size), PSUM fp32.
  * PV uses expS^T tiles as the STATIONARY and natural [V|1] as the moving
    ([128 k, 65]): output is ctx in natural [q, d] orientation with the
    softmax denominator in column 64 -> no output transposes, and PV costs
    65 rows per (k-tile, q-tile) instead of 512 per (k-tile, 512q).
  * exp runs on ACT in [128, 1024] chunks (double-buffered PSUM); ACT is
    the global bottleneck (~267us busy), so the schedule keeps it saturated:
    - early start: attention chunk (qc0,h0) k-tiles 0..7 begin right after
      hs-chunks 0,1 are projected; hs-chunks 2,3 interleave into those
      iterations, so the first exp fires ~35us earlier than a sequential
      phase-A/phase-C split;
    - batch 1's entire projection phase is interleaved into batch 0's
      attention loop (~2 PE instructions per k-tile iteration);
    - PV for chunk (qc,h) is software-pipelined into the scores/exp loop of
      the next chunk so ctx accumulation groups stay sequential per PSUM
      zero region (hardware allows one open group per 2KB bank).
"""

import sys
from contextlib import ExitStack

import numpy as np

sys.path.insert(0, "/opt/trn_rl_repo")

import concourse.bass as bass  # noqa: E402
import concourse.bacc as bacc  # noqa: E402
import concourse.tile as tile  # noqa: E402
from concourse import mybir  # noqa: E402
from concourse.bass_utils import run_bass_kernel_spmd  # noqa: E402

B = 2
S = 2048
D = 2048
HD = 64
NCORES = 8
QH = 4           # q heads per core
MCOLS = QH * HD  # 256 output cols per core

BF = mybir.dt.bfloat16
F32 = mybir.dt.float32
Exp = mybir.ActivationFunctionType.Exp

NDT = 16         # d tiles of 128
NSC = 4          # s chunks of 512 per batch
NKT = 16         # s_k tiles of 128
NQC = 2          # q chunks of 1024 per batch
QTPC = 8         # q tiles of 128 per q chunk


def build_nc():
    nc = bacc.Bacc("TRN2", target_bir_lowering=False, debug=False)

    hsr_d = nc.dram_tensor("hsr", [B, NSC, 128, NDT * 512], BF,
                           kind="ExternalInput")
    wqr_d = nc.dram_tensor("wqr", [128, NDT * 256], BF, kind="ExternalInput")
    wkvr_d = nc.dram_tensor("wkvr", [128, NDT * 128], BF, kind="ExternalInput")
    bq_d = nc.dram_tensor("bq", [128, 2], F32, kind="ExternalInput")
    bkv_d = nc.dram_tensor("bkv", [128, 1], F32, kind="ExternalInput")
    id_d = nc.dram_tensor("ident", [128, 128], BF, kind="ExternalInput")
    out_d = nc.dram_tensor("out", [B, S, MCOLS], F32, kind="ExternalOutput")

    with tile.TileContext(nc) as tc, ExitStack() as ctx:
        const = ctx.enter_context(tc.tile_pool(name="const", bufs=1))
        wqp = ctx.enter_context(tc.tile_pool(name="wqp", bufs=1))
        hstp = ctx.enter_context(tc.tile_pool(name="hstp", bufs=3))
        qtp = ctx.enter_context(tc.tile_pool(name="qtp", bufs=4))
        kvp = ctx.enter_context(tc.tile_pool(name="kvp", bufs=2))
        kthp = ctx.enter_context(tc.tile_pool(name="kthp", bufs=2))
        v1p = ctx.enter_context(tc.tile_pool(name="v1p", bufs=2 * NKT))
        expp = ctx.enter_context(tc.tile_pool(name="expp", bufs=33))
        recp = ctx.enter_context(tc.tile_pool(name="recp", bufs=4))
        outp = ctx.enter_context(tc.tile_pool(name="outp", bufs=16))
        psap = ctx.enter_context(tc.tile_pool(name="psap", bufs=2, space="PSUM"))
        pssp = ctx.enter_context(tc.tile_pool(name="pssp", bufs=2, space="PSUM"))
        ctxp = ctx.enter_context(tc.tile_pool(name="ctxp", bufs=1, space="PSUM"))

        ident = const.tile([128, 128], BF, tag="ident")
        nc.sync.dma_start(out=ident[:], in_=id_d[:])
        bq_sb = const.tile([128, 2], F32, tag="bq")
        nc.sync.dma_start(out=bq_sb[:], in_=bq_d[:])
        bkv_sb = const.tile([128, 1], F32, tag="bkv")
        nc.sync.dma_start(out=bkv_sb[:], in_=bkv_d[:])
        zb = const.tile([128, 1], F32, tag="zb")
        nc.vector.memset(zb[:], 0.0)

        wq_sb = wqp.tile([128, NDT * 256], BF, tag="wq")
        nc.sync.dma_start(out=wq_sb[:], in_=wqr_d[:])
        wkv_sb = wqp.tile([128, NDT * 128], BF, tag="wkv")
        nc.sync.dma_start(out=wkv_sb[:], in_=wkvr_d[:])

        # hsT chunk loads for both batches, issued upfront (slot reuse
        # dependencies serialize them against phase-A consumption).
        hst = [[None] * NSC for _ in range(B)]
        for b in range(B):
            for sc in range(NSC):
                h = hstp.tile([128, NDT * 512], BF, tag="hst",
                              name=f"hst{b}_{sc}")
                nc.sync.dma_start(out=h[:], in_=hsr_d[b, sc])
                hst[b][sc] = h

        qT = [[None, None] for _ in range(B)]
        kvT = [None] * B
        kth = [None] * B
        # V tiles pre-created with their ones column set while the DMAs of
        # the first hs chunks are still in flight (DVE is idle then).
        v1 = [[None] * NKT for _ in range(B)]
        for b in range(B):
            for kt in range(NKT):
                v = v1p.tile([128, 65], BF, tag="v1", name=f"v1_{b}_{kt}")
                nc.vector.memset(v[:, 64:65], 1.0)
                v1[b][kt] = v

        def init_b(b):
            qT[b][0] = qtp.tile([128, S], BF, tag="qt", name=f"qT{b}_0")
            qT[b][1] = qtp.tile([128, S], BF, tag="qt", name=f"qT{b}_1")
            kvT[b] = kvp.tile([128, S], BF, tag="kv", name=f"kvT{b}")
            kth[b] = kthp.tile([128, S], BF, tag="kth", name=f"kth{b}")

        def chunk_units(b, sc):
            """Projections + V tiles + partial kth copy for hs chunk sc of
            batch b; one engine instruction per yield."""
            hs_t = hst[b][sc]
            c0 = sc * 512
            for qc in range(2):
                ps = psap.tile([128, 512], F32, tag="ps")
                for dt_ in range(NDT):
                    nc.tensor.matmul(
                        ps[:],
                        wq_sb[:, dt_ * 256 + qc * 128:
                              dt_ * 256 + (qc + 1) * 128],
                        hs_t[:, dt_ * 512:(dt_ + 1) * 512],
                        start=(dt_ == 0), stop=(dt_ == NDT - 1),
                    )
                    yield
                nc.vector.tensor_scalar_add(
                    qT[b][qc][:, c0:c0 + 512], ps[:], bq_sb[:, qc:qc + 1])
                yield
            ps = psap.tile([128, 512], F32, tag="ps")
            for dt_ in range(NDT):
                nc.tensor.matmul(
                    ps[:], wkv_sb[:, dt_ * 128:(dt_ + 1) * 128],
                    hs_t[:, dt_ * 512:(dt_ + 1) * 512],
                    start=(dt_ == 0), stop=(dt_ == NDT - 1),
                )
                yield
            nc.vector.tensor_scalar_add(
                kvT[b][:, c0:c0 + 512], ps[:], bkv_sb[:])
            yield
            # K^T rows shifted to partitions 64:127 for odd heads; issued
            # from the Pool queue so the SP queue (hsT loads) is not blocked.
            nc.gpsimd.dma_start(out=kth[b][64:128, c0:c0 + 512],
                                in_=kvT[b][0:64, c0:c0 + 512])
            yield
            # V natural tiles [s_k 128, 64]; the ones column was pre-set.
            for kt in range(sc * 4, sc * 4 + 4):
                pst = psap.tile([128, 512], BF, tag="ps", name=f"pst{b}_{kt}")
                nc.tensor.transpose(
                    pst[:, 0:64], kvT[b][64:128, kt * 128:(kt + 1) * 128],
                    ident[64:128, 64:128],
                )
                yield
                nc.vector.tensor_copy(v1[b][kt][:, 0:64], pst[:, 0:64])
                yield

        def gen_a(b):
            init_b(b)
            for sc in range(NSC):
                yield from chunk_units(b, sc)

        # ---- phase C machinery ----
        outt = {0: [None] * (NQC * QTPC), 1: [None] * (NQC * QTPC)}
        exs = {}
        ctxs = {}

        def coff(qi):
            # qi 0..6 packed in bank 0; qi 7 at the bank-1 boundary so no
            # accumulation group straddles a PSUM bank.
            return qi * 65 if qi < 7 else 512

        def pv_chunk(bkey, it):
            b, qc, h = bkey
            ctx_t = ctxs[bkey]
            qi = it // 2
            base = (it % 2) * 8
            for k2 in range(8):
                kt = base + k2
                nc.tensor.matmul(
                    ctx_t[:, coff(qi):coff(qi) + 65],
                    exs[bkey][kt][:, qi * 128:(qi + 1) * 128],
                    v1[b][kt][:],
                    start=(it % 2 == 0 and k2 == 0),
                    stop=(it % 2 == 1 and k2 == 7),
                )

        def normalize_qi(bkey, qi):
            b, qc, h = bkey
            ctx_t = ctxs[bkey]
            qtile = qc * QTPC + qi
            if h == 0:
                outt[b][qtile] = outp.tile([128, MCOLS], F32, tag="out",
                                           name=f"outt{b}_{qtile}")
            rec = recp.tile([128, 1], F32, tag="rec")
            nc.vector.reciprocal(
                rec[:], ctx_t[:, coff(qi) + 64:coff(qi) + 65])
            nc.vector.tensor_scalar_mul(
                outt[b][qtile][:, h * 64:(h + 1) * 64],
                ctx_t[:, coff(qi):coff(qi) + 64], rec[:])
            if h == QH - 1:
                nc.sync.dma_start(
                    out=out_d[b, qtile * 128:(qtile + 1) * 128, :],
                    in_=outt[b][qtile][:])

        def begin_chunk(bkey):
            b, qc, h = bkey
            ctxs[bkey] = ctxp.tile([128, 577], F32, tag="ctx",
                                   name=f"ctx{b}_{qc}_{h}")
            exs[bkey] = []

        def score_exp(bkey, kt):
            b, qc, h = bkey
            r0 = (h % 2) * 64
            kmat = kvT[b] if r0 == 0 else kth[b]
            qt = qT[b][h // 2]
            pss = pssp.tile([128, 1024], F32, tag="pss")
            for j in range(2):
                q0 = qc * 1024 + j * 512
                nc.tensor.matmul(
                    pss[:, j * 512:(j + 1) * 512],
                    kmat[r0:r0 + 64, kt * 128:(kt + 1) * 128],
                    qt[r0:r0 + 64, q0:q0 + 512],
                    start=True, stop=True,
                )
            ex = expp.tile([128, 1024], BF, tag="ex")
            nc.scalar.activation(ex[:], pss[:], Exp, bias=zb[:])
            exs[bkey].append(ex)

        def finish_chunk(bkey):
            del exs[bkey]
            del ctxs[bkey]

        def advance(gen, n):
            if gen is None:
                return None
            for _ in range(n):
                if next(gen, StopIteration) is StopIteration:
                    return None
            return gen

        # ---- emission ----
        # Batch 0 prologue: project hs chunks 0,1; run (qc0,h0) k-tiles 0..7
        # while chunk 2 interleaves; k-tiles 8..11 with chunk 3; 12..15 clean.
        init_b(0)
        for _ in chunk_units(0, 0):
            pass
        for _ in chunk_units(0, 1):
            pass
        key00 = (0, 0, 0)
        begin_chunk(key00)
        g2 = chunk_units(0, 2)
        g3 = chunk_units(0, 3)
        for kt in range(NKT):
            if kt == 8 and g2 is not None:
                for _ in g2:
                    pass
                g2 = None
            if kt == 12 and g3 is not None:
                for _ in g3:
                    pass
                g3 = None
            score_exp(key00, kt)
            if kt < 8:
                g2 = advance(g2, 7)
            elif kt < 12:
                g3 = advance(g3, 13)

        # Steady state: remaining 7 chunks of batch 0 with batch 1's
        # projections interleaved, then batch 1's chunks.
        order0 = [(0, qc, h) for qc in range(NQC) for h in range(QH)][1:]
        order1 = [(1, qc, h) for qc in range(NQC) for h in range(QH)]
        g1 = gen_a(1)
        prev = key00
        for bkey in order0:
            begin_chunk(bkey)
            for kt in range(NKT):
                score_exp(bkey, kt)
                pv_chunk(prev, kt)
                if kt % 2 == 1:
                    normalize_qi(prev, kt // 2)
                g1 = advance(g1, 2)
            finish_chunk(prev)
            prev = bkey
        # flush batch 1 projections before its attention begins
        if g1 is not None:
            for _ in g1:
                pass
        for bkey in order1:
            begin_chunk(bkey)
            for kt in range(NKT):
                score_exp(bkey, kt)
                pv_chunk(prev, kt)
                if kt % 2 == 1:
                    normalize_qi(prev, kt // 2)
            finish_chunk(prev)
            prev = bkey
        for it in range(NKT):
            pv_chunk(prev, it)
            if it % 2 == 1:
                normalize_qi(prev, it // 2)
        finish_chunk(prev)

    nc.compile()
    return nc


def make_in_maps(hidden_states, Wq, bq, Wk, bk, Wv, bv):
    bf = mybir.dt.np(BF)
    hs = np.asarray(hidden_states, dtype=np.float32)
    Wq = np.asarray(Wq, dtype=np.float32)
    bq = np.asarray(bq, dtype=np.float32)
    Wk = np.asarray(Wk, dtype=np.float32)
    bk = np.asarray(bk, dtype=np.float32)
    Wv = np.asarray(Wv, dtype=np.float32)
    bv = np.asarray(bv, dtype=np.float32)
    sc = 1.0 / np.sqrt(np.float32(HD))
    # [b, sc, p(d in tile), t(d tile), j(s in chunk)] -> [2, 4, 128, 8192]
    hsr = np.ascontiguousarray(
        hs.reshape(B, NSC, 512, NDT, 128).transpose(0, 1, 4, 3, 2)
        .reshape(B, NSC, 128, NDT * 512).astype(bf))
    ident = np.eye(128, dtype=bf)
    in_maps = []
    for c in range(NCORES):
        qs = slice(c * MCOLS, (c + 1) * MCOLS)
        ks = slice(c * HD, (c + 1) * HD)
        wqs = (Wq[:, qs] * sc).astype(bf)
        wqr = np.ascontiguousarray(
            wqs.reshape(NDT, 128, MCOLS).transpose(1, 0, 2)
            .reshape(128, NDT * MCOLS))
        wkvs = np.concatenate([Wk[:, ks], Wv[:, ks]], axis=1).astype(bf)
        wkvr = np.ascontiguousarray(
            wkvs.reshape(NDT, 128, 128).transpose(1, 0, 2)
            .reshape(128, NDT * 128))
        bq_c = np.ascontiguousarray((bq[qs] * sc).reshape(2, 128).T)
        in_maps.append({
            "hsr": hsr,
            "wqr": wqr,
            "wkvr": wkvr,
            "bq": bq_c,
            "bkv": np.concatenate([bk[ks], bv[ks]]).reshape(128, 1),
            "ident": ident,
        })
    return in_maps


_NC_CACHE = {}


def get_nc():
    if "nc" not in _NC_CACHE:
        _NC_CACHE["nc"] = build_nc()
    return _NC_CACHE["nc"]


def kernel(hidden_states, Wq, bq, Wk, bk, Wv, bv):
    nc = get_nc()
    in_maps = make_in_maps(hidden_states, Wq, bq, Wk, bk, Wv, bv)
    res = run_bass_kernel_spmd(nc, in_maps, list(range(NCORES)))
    outs = [np.asarray(r["out"], dtype=np.float32) for r in res.results]
    return np.concatenate(outs, axis=-1)


# revision 14
# speedup vs baseline: 1.7876x; 1.0100x over previous
"""Trainium2 Bass kernel for GroupedQueryAttention (v3, bf16, early-start).

Sharding: 8 cores; core c owns KV head g=c and Q heads 4c..4c+3, both batch
elements. Each core computes its [2, 2048, 256] output slice; host concats.

Design:
  * hs is pre-transposed AND pre-cast to bf16 on the host (hsr layout:
    [b, s-chunk, d-in-tile(128), d-tile(16) x s(512)]), removing all 512
    PE transposes and their DVE drains from the device program.
  * All PE operands are bf16 (1.0 cycles/row at any ap size), PSUM fp32.
  * PV uses expS^T tiles as the STATIONARY and natural [V|1] as the moving
    ([128 k, 65]): output is ctx in natural [q, d] orientation with the
    softmax denominator in column 64 -> no output transposes, and PV costs
    65 rows per (k-tile, q-tile) instead of 512 per (k-tile, 512q).
  * exp runs on ACT in [128, 1024] chunks (double-buffered PSUM); ACT is
    the global bottleneck (~267us busy), so the schedule keeps it saturated:
    - early start: attention chunk (qc0,h0) k-tiles 0..7 begin right after
      hs-chunks 0,1 are projected; hs-chunks 2,3 interleave into those
      iterations, so the first exp fires ~35us earlier than a sequential
      phase-A/phase-C split;
    - batch 1's entire projection phase is interleaved into batch 0's
      attention loop (~2 PE instructions per k-tile iteration);
    - PV for chunk (qc,h) is software-pipelined into the scores/exp loop of
      the next chunk so ctx accumulation groups stay sequential per PSUM
      zero region (hardware allows one open group per 2KB bank).
"""

import sys
from contextlib import ExitStack

import numpy as np

sys.path.insert(0, "/opt/trn_rl_repo")

import concourse.bass as bass  # noqa: E402
import concourse.bacc as bacc  # noqa: E402
import concourse.tile as tile  # noqa: E402
from concourse import mybir  # noqa: E402
from concourse.bass_utils import run_bass_kernel_spmd  # noqa: E402

B = 2
S = 2048
D = 2048
HD = 64
NCORES = 8
QH = 4           # q heads per core
MCOLS = QH * HD  # 256 output cols per core

BF = mybir.dt.bfloat16
F32 = mybir.dt.float32
Exp = mybir.ActivationFunctionType.Exp

NDT = 16         # d tiles of 128
NSC = 4          # s chunks of 512 per batch
NKT = 16         # s_k tiles of 128
NQC = 2          # q chunks of 1024 per batch
QTPC = 8         # q tiles of 128 per q chunk


def build_nc():
    nc = bacc.Bacc("TRN2", target_bir_lowering=False, debug=False)

    hsr_d = nc.dram_tensor("hsr", [B, NSC, 128, NDT * 512], BF,
                           kind="ExternalInput")
    wqr_d = nc.dram_tensor("wqr", [128, NDT * 256], BF, kind="ExternalInput")
    wkvr_d = nc.dram_tensor("wkvr", [128, NDT * 128], BF, kind="ExternalInput")
    bq_d = nc.dram_tensor("bq", [128, 2], F32, kind="ExternalInput")
    bkv_d = nc.dram_tensor("bkv", [128, 1], F32, kind="ExternalInput")
    id_d = nc.dram_tensor("ident", [128, 128], BF, kind="ExternalInput")
    out_d = nc.dram_tensor("out", [B, S, MCOLS], F32, kind="ExternalOutput")

    with tile.TileContext(nc) as tc, ExitStack() as ctx:
        const = ctx.enter_context(tc.tile_pool(name="const", bufs=1))
        wqp = ctx.enter_context(tc.tile_pool(name="wqp", bufs=1))
        hstp = ctx.enter_context(tc.tile_pool(name="hstp", bufs=3))
        qtp = ctx.enter_context(tc.tile_pool(name="qtp", bufs=4))
        kvp = ctx.enter_context(tc.tile_pool(name="kvp", bufs=2))
        kthp = ctx.enter_context(tc.tile_pool(name="kthp", bufs=2))
        v1p = ctx.enter_context(tc.tile_pool(name="v1p", bufs=2 * NKT))
        expp = ctx.enter_context(tc.tile_pool(name="expp", bufs=33))
        recp = ctx.enter_context(tc.tile_pool(name="recp", bufs=4))
        outp = ctx.enter_context(tc.tile_pool(name="outp", bufs=16))
        psap = ctx.enter_context(tc.tile_pool(name="psap", bufs=2, space="PSUM"))
        pssp = ctx.enter_context(tc.tile_pool(name="pssp", bufs=2, space="PSUM"))
        ctxp = ctx.enter_context(tc.tile_pool(name="ctxp", bufs=1, space="PSUM"))

        # DMA order is the cold-start critical path: Wq first, then hs chunk
        # 0, so the first projection chain can begin ~9us in; everything else
        # follows.
        wq_sb = wqp.tile([128, NDT * 256], BF, tag="wq")
        nc.sync.dma_start(out=wq_sb[:], in_=wqr_d[:])
        hst = [[None] * NSC for _ in range(B)]
        for b in range(B):
            for sc in range(NSC):
                hst[b][sc] = hstp.tile([128, NDT * 512], BF, tag="hst",
                                       name=f"hst{b}_{sc}")
        nc.sync.dma_start(out=hst[0][0][:], in_=hsr_d[0, 0])
        wkv_sb = wqp.tile([128, NDT * 128], BF, tag="wkv")
        nc.sync.dma_start(out=wkv_sb[:], in_=wkvr_d[:])
        ident = const.tile([128, 128], BF, tag="ident")
        nc.sync.dma_start(out=ident[:], in_=id_d[:])
        bq_sb = const.tile([128, 2], F32, tag="bq")
        nc.sync.dma_start(out=bq_sb[:], in_=bq_d[:])
        bkv_sb = const.tile([128, 1], F32, tag="bkv")
        nc.sync.dma_start(out=bkv_sb[:], in_=bkv_d[:])
        zb = const.tile([128, 1], F32, tag="zb")
        nc.vector.memset(zb[:], 0.0)
        for b in range(B):
            for sc in range(NSC):
                if (b, sc) == (0, 0):
                    continue
                nc.sync.dma_start(out=hst[b][sc][:], in_=hsr_d[b, sc])

        qT = [[None, None] for _ in range(B)]
        kvT = [None] * B
        kth = [None] * B
        # V tiles pre-created with their ones column set while the DMAs of
        # the first hs chunks are still in flight (DVE is idle then).
        v1 = [[None] * NKT for _ in range(B)]
        for b in range(B):
            for kt in range(NKT):
                v = v1p.tile([128, 65], BF, tag="v1", name=f"v1_{b}_{kt}")
                nc.vector.memset(v[:, 64:65], 1.0)
                v1[b][kt] = v

        def init_b(b):
            qT[b][0] = qtp.tile([128, S], BF, tag="qt", name=f"qT{b}_0")
            qT[b][1] = qtp.tile([128, S], BF, tag="qt", name=f"qT{b}_1")
            kvT[b] = kvp.tile([128, S], BF, tag="kv", name=f"kvT{b}")
            kth[b] = kthp.tile([128, S], BF, tag="kth", name=f"kth{b}")

        def chunk_units(b, sc):
            """Projections + V tiles + partial kth copy for hs chunk sc of
            batch b; one engine instruction per yield."""
            hs_t = hst[b][sc]
            c0 = sc * 512
            for qc in range(2):
                ps = psap.tile([128, 512], F32, tag="ps")
                for dt_ in range(NDT):
                    nc.tensor.matmul(
                        ps[:],
                        wq_sb[:, dt_ * 256 + qc * 128:
                              dt_ * 256 + (qc + 1) * 128],
                        hs_t[:, dt_ * 512:(dt_ + 1) * 512],
                        start=(dt_ == 0), stop=(dt_ == NDT - 1),
                    )
                    yield
                nc.vector.tensor_scalar_add(
                    qT[b][qc][:, c0:c0 + 512], ps[:], bq_sb[:, qc:qc + 1])
                yield
            ps = psap.tile([128, 512], F32, tag="ps")
            for dt_ in range(NDT):
                nc.tensor.matmul(
                    ps[:], wkv_sb[:, dt_ * 128:(dt_ + 1) * 128],
                    hs_t[:, dt_ * 512:(dt_ + 1) * 512],
                    start=(dt_ == 0), stop=(dt_ == NDT - 1),
                )
                yield
            nc.vector.tensor_scalar_add(
                kvT[b][:, c0:c0 + 512], ps[:], bkv_sb[:])
            yield
            # K^T rows shifted to partitions 64:127 for odd heads; issued
            # from the Pool queue so the SP queue (hsT loads) is not blocked.
            nc.gpsimd.dma_start(out=kth[b][64:128, c0:c0 + 512],
                                in_=kvT[b][0:64, c0:c0 + 512])
            yield
            # V natural tiles [s_k 128, 64]; the ones column was pre-set.
            for kt in range(sc * 4, sc * 4 + 4):
                pst = psap.tile([128, 512], BF, tag="ps", name=f"pst{b}_{kt}")
                nc.tensor.transpose(
                    pst[:, 0:64], kvT[b][64:128, kt * 128:(kt + 1) * 128],
                    ident[64:128, 64:128],
                )
                yield
                nc.vector.tensor_copy(v1[b][kt][:, 0:64], pst[:, 0:64])
                yield

        def gen_a(b):
            init_b(b)
            for sc in range(NSC):
                yield from chunk_units(b, sc)

        # ---- phase C machinery ----
        outt = {0: [None] * (NQC * QTPC), 1: [None] * (NQC * QTPC)}
        exs = {}
        ctxs = {}

        def coff(qi):
            # qi 0..6 packed in bank 0; qi 7 at the bank-1 boundary so no
            # accumulation group straddles a PSUM bank.
            return qi * 65 if qi < 7 else 512

        def pv_chunk(bkey, it, swap=False):
            # it 0..15: two passes of 8 k-tiles per q-tile qi = it//2.
            # swap=True consumes k-tiles 8..15 on the first pass (used when
            # the producing chunk emitted its exps in swapped order).
            b, qc, h = bkey
            ctx_t = ctxs[bkey]
            qi = it // 2
            base = (it % 2) * 8
            if swap:
                base = 8 - base
            for k2 in range(8):
                kt = base + k2
                nc.tensor.matmul(
                    ctx_t[:, coff(qi):coff(qi) + 65],
                    exs[bkey][kt][:, qi * 128:(qi + 1) * 128],
                    v1[b][kt][:],
                    start=(it % 2 == 0 and k2 == 0),
                    stop=(it % 2 == 1 and k2 == 7),
                )

        def normalize_qi(bkey, qi):
            b, qc, h = bkey
            ctx_t = ctxs[bkey]
            qtile = qc * QTPC + qi
            if h == 0:
                outt[b][qtile] = outp.tile([128, MCOLS], F32, tag="out",
                                           name=f"outt{b}_{qtile}")
            rec = recp.tile([128, 1], F32, tag="rec")
            nc.vector.reciprocal(
                rec[:], ctx_t[:, coff(qi) + 64:coff(qi) + 65])
            nc.vector.tensor_scalar_mul(
                outt[b][qtile][:, h * 64:(h + 1) * 64],
                ctx_t[:, coff(qi):coff(qi) + 64], rec[:])
            if h == QH - 1:
                nc.sync.dma_start(
                    out=out_d[b, qtile * 128:(qtile + 1) * 128, :],
                    in_=outt[b][qtile][:])

        def begin_chunk(bkey):
            b, qc, h = bkey
            ctxs[bkey] = ctxp.tile([128, 577], F32, tag="ctx",
                                   name=f"ctx{b}_{qc}_{h}")
            exs[bkey] = []

        def score_exp(bkey, kt):
            b, qc, h = bkey
            r0 = (h % 2) * 64
            kmat = kvT[b] if r0 == 0 else kth[b]
            qt = qT[b][h // 2]
            pss = pssp.tile([128, 1024], F32, tag="pss")
            for j in range(2):
                q0 = qc * 1024 + j * 512
                nc.tensor.matmul(
                    pss[:, j * 512:(j + 1) * 512],
                    kmat[r0:r0 + 64, kt * 128:(kt + 1) * 128],
                    qt[r0:r0 + 64, q0:q0 + 512],
                    start=True, stop=True,
                )
            ex = expp.tile([128, 1024], BF, tag="ex")
            nc.scalar.activation(ex[:], pss[:], Exp, bias=zb[:])
            while len(exs[bkey]) <= kt:
                exs[bkey].append(None)
            exs[bkey][kt] = ex

        def finish_chunk(bkey):
            del exs[bkey]
            del ctxs[bkey]

        def advance(gen, n):
            if gen is None:
                return None
            for _ in range(n):
                if next(gen, StopIteration) is StopIteration:
                    return None
            return gen

        # ---- emission ----
        # Batch 0 prologue: project hs chunks 0,1; run (qc0,h0) k-tiles 0..7
        # while chunk 2 interleaves; k-tiles 8..11 with chunk 3; 12..15 clean.
        init_b(0)
        for _ in chunk_units(0, 0):
            pass
        for _ in chunk_units(0, 1):
            pass
        key00 = (0, 0, 0)
        begin_chunk(key00)
        g2 = chunk_units(0, 2)
        g3 = chunk_units(0, 3)
        for kt in range(NKT):
            if kt == 8 and g2 is not None:
                for _ in g2:
                    pass
                g2 = None
            if kt == 12 and g3 is not None:
                for _ in g3:
                    pass
                g3 = None
            score_exp(key00, kt)
            if kt < 8:
                g2 = advance(g2, 7)
            elif kt < 12:
                g3 = advance(g3, 13)

        # Steady state: remaining 7 chunks of batch 0 with batch 1's
        # projections interleaved, then batch 1's chunks.
        order0 = [(0, qc, h) for qc in range(NQC) for h in range(QH)][1:]
        order1 = [(1, qc, h) for qc in range(NQC) for h in range(QH)]
        g1 = gen_a(1)
        prev = key00
        for bkey in order0:
            begin_chunk(bkey)
            for kt in range(NKT):
                score_exp(bkey, kt)
                pv_chunk(prev, kt)
                if kt % 2 == 1:
                    normalize_qi(prev, kt // 2)
                g1 = advance(g1, 2)
            finish_chunk(prev)
            prev = bkey
        # flush batch 1 projections before its attention begins
        if g1 is not None:
            for _ in g1:
                pass
        for bkey in order1:
            last = bkey == order1[-1]
            begin_chunk(bkey)
            for i, kt in enumerate(range(NKT)):
                # The final chunk emits k-tiles 8..15 first so its PV flush
                # (which consumes the late k-tiles on odd passes) is never
                # waiting on the exp backlog at the very end.
                score_exp(bkey, (kt + 8) % NKT if last else kt)
                pv_chunk(prev, kt)
                if kt % 2 == 1:
                    normalize_qi(prev, kt // 2)
            finish_chunk(prev)
            prev = bkey
        for it in range(NKT):
            pv_chunk(prev, it, swap=True)
            if it % 2 == 1:
                normalize_qi(prev, it // 2)
        finish_chunk(prev)

    nc.compile()
    return nc


def make_in_maps(hidden_states, Wq, bq, Wk, bk, Wv, bv):
    bf = mybir.dt.np(BF)
    hs = np.asarray(hidden_states, dtype=np.float32)
    Wq = np.asarray(Wq, dtype=np.float32)
    bq = np.asarray(bq, dtype=np.float32)
    Wk = np.asarray(Wk, dtype=np.float32)
    bk = np.asarray(bk, dtype=np.float32)
    Wv = np.asarray(Wv, dtype=np.float32)
    bv = np.asarray(bv, dtype=np.float32)
    sc = 1.0 / np.sqrt(np.float32(HD))
    # [b, sc, p(d in tile), t(d tile), j(s in chunk)] -> [2, 4, 128, 8192]
    hsr = np.ascontiguousarray(
        hs.reshape(B, NSC, 512, NDT, 128).transpose(0, 1, 4, 3, 2)
        .reshape(B, NSC, 128, NDT * 512).astype(bf))
    ident = np.eye(128, dtype=bf)
    in_maps = []
    for c in range(NCORES):
        qs = slice(c * MCOLS, (c + 1) * MCOLS)
        ks = slice(c * HD, (c + 1) * HD)
        wqs = (Wq[:, qs] * sc).astype(bf)
        wqr = np.ascontiguousarray(
            wqs.reshape(NDT, 128, MCOLS).transpose(1, 0, 2)
            .reshape(128, NDT * MCOLS))
        wkvs = np.concatenate([Wk[:, ks], Wv[:, ks]], axis=1).astype(bf)
        wkvr = np.ascontiguousarray(
            wkvs.reshape(NDT, 128, 128).transpose(1, 0, 2)
            .reshape(128, NDT * 128))
        bq_c = np.ascontiguousarray((bq[qs] * sc).reshape(2, 128).T)
        in_maps.append({
            "hsr": hsr,
            "wqr": wqr,
            "wkvr": wkvr,
            "bq": bq_c,
            "bkv": np.concatenate([bk[ks], bv[ks]]).reshape(128, 1),
            "ident": ident,
        })
    return in_maps


_NC_CACHE = {}


def get_nc():
    if "nc" not in _NC_CACHE:
        _NC_CACHE["nc"] = build_nc()
    return _NC_CACHE["nc"]


def kernel(hidden_states, Wq, bq, Wk, bk, Wv, bv):
    nc = get_nc()
    in_maps = make_in_maps(hidden_states, Wq, bq, Wk, bk, Wv, bv)
    res = run_bass_kernel_spmd(nc, in_maps, list(range(NCORES)))
    outs = [np.asarray(r["out"], dtype=np.float32) for r in res.results]
    return np.concatenate(outs, axis=-1)


# revision 19
# speedup vs baseline: 1.8028x; 1.0085x over previous
"""Trainium2 Bass kernel for GroupedQueryAttention (v3, bf16, early-start).

Sharding: 8 cores; core c owns KV head g=c and Q heads 4c..4c+3, both batch
elements. Each core computes its [2, 2048, 256] output slice; host concats.

Design:
  * hs is pre-transposed AND pre-cast to bf16 on the host (hsr layout:
    [b, s-chunk, d-in-tile(128), d-tile(16) x s(512)]), removing all 512
    PE transposes and their DVE drains from the device program.
  * All PE operands are bf16 (1.0 cycles/row at any ap size), PSUM fp32.
  * PV uses expS^T tiles as the STATIONARY and natural [V|1] as the moving
    ([128 k, 65]): output is ctx in natural [q, d] orientation with the
    softmax denominator in column 64 -> no output transposes, and PV costs
    65 rows per (k-tile, q-tile) instead of 512 per (k-tile, 512q).
  * exp runs on ACT in [128, 1024] chunks (double-buffered PSUM); ACT is
    the global bottleneck (~267us busy), so the schedule keeps it saturated:
    - early start: attention chunk (qc0,h0) k-tiles 0..7 begin right after
      hs-chunks 0,1 are projected; hs-chunks 2,3 interleave into those
      iterations, so the first exp fires ~35us earlier than a sequential
      phase-A/phase-C split;
    - batch 1's entire projection phase is interleaved into batch 0's
      attention loop (~2 PE instructions per k-tile iteration);
    - PV for chunk (qc,h) is software-pipelined into the scores/exp loop of
      the next chunk so ctx accumulation groups stay sequential per PSUM
      zero region (hardware allows one open group per 2KB bank).
"""

import sys
from contextlib import ExitStack

import numpy as np

sys.path.insert(0, "/opt/trn_rl_repo")

import concourse.bass as bass  # noqa: E402
import concourse.bacc as bacc  # noqa: E402
import concourse.tile as tile  # noqa: E402
from concourse import mybir  # noqa: E402
from concourse.bass_utils import run_bass_kernel_spmd  # noqa: E402

B = 2
S = 2048
D = 2048
HD = 64
NCORES = 8
QH = 4           # q heads per core
MCOLS = QH * HD  # 256 output cols per core

BF = mybir.dt.bfloat16
F32 = mybir.dt.float32
Exp = mybir.ActivationFunctionType.Exp

NDT = 16         # d tiles of 128
NSC = 4          # s chunks of 512 per batch
NKT = 16         # s_k tiles of 128
NQC = 2          # q chunks of 1024 per batch
QTPC = 8         # q tiles of 128 per q chunk


def build_nc():
    nc = bacc.Bacc("TRN2", target_bir_lowering=False, debug=False)

    hsr_d = nc.dram_tensor("hsr", [B, NSC, 128, NDT * 512], BF,
                           kind="ExternalInput")
    wqr_d = nc.dram_tensor("wqr", [128, NDT * 256], BF, kind="ExternalInput")
    wkvr_d = nc.dram_tensor("wkvr", [128, NDT * 128], BF, kind="ExternalInput")
    bq_d = nc.dram_tensor("bq", [128, 2], F32, kind="ExternalInput")
    bkv_d = nc.dram_tensor("bkv", [128, 1], F32, kind="ExternalInput")
    id_d = nc.dram_tensor("ident", [128, 128], BF, kind="ExternalInput")
    out_d = nc.dram_tensor("out", [B, S, MCOLS], F32, kind="ExternalOutput")

    with tile.TileContext(nc) as tc, ExitStack() as ctx:
        const = ctx.enter_context(tc.tile_pool(name="const", bufs=1))
        wqp = ctx.enter_context(tc.tile_pool(name="wqp", bufs=1))
        hstp = ctx.enter_context(tc.tile_pool(name="hstp", bufs=3))
        qtp = ctx.enter_context(tc.tile_pool(name="qtp", bufs=4))
        kvp = ctx.enter_context(tc.tile_pool(name="kvp", bufs=2))
        kthp = ctx.enter_context(tc.tile_pool(name="kthp", bufs=2))
        v1p = ctx.enter_context(tc.tile_pool(name="v1p", bufs=2 * NKT))
        expp = ctx.enter_context(tc.tile_pool(name="expp", bufs=38))
        recp = ctx.enter_context(tc.tile_pool(name="recp", bufs=4))
        outp = ctx.enter_context(tc.tile_pool(name="outp", bufs=16))
        psap = ctx.enter_context(tc.tile_pool(name="psap", bufs=2, space="PSUM"))
        pssp = ctx.enter_context(tc.tile_pool(name="pssp", bufs=2, space="PSUM"))
        ctxp = ctx.enter_context(tc.tile_pool(name="ctxp", bufs=1, space="PSUM"))

        # DMA order is the cold-start critical path: Wq first, then hs chunk
        # 0, so the first projection chain can begin ~9us in; everything else
        # follows.
        wq_sb = wqp.tile([128, NDT * 256], BF, tag="wq")
        nc.sync.dma_start(out=wq_sb[:], in_=wqr_d[:])
        hst = [[None] * NSC for _ in range(B)]
        for b in range(B):
            for sc in range(NSC):
                hst[b][sc] = hstp.tile([128, NDT * 512], BF, tag="hst",
                                       name=f"hst{b}_{sc}")
        nc.sync.dma_start(out=hst[0][0][:], in_=hsr_d[0, 0])
        wkv_sb = wqp.tile([128, NDT * 128], BF, tag="wkv")
        nc.sync.dma_start(out=wkv_sb[:], in_=wkvr_d[:])
        ident = const.tile([128, 128], BF, tag="ident")
        nc.sync.dma_start(out=ident[:], in_=id_d[:])
        bq_sb = const.tile([128, 2], F32, tag="bq")
        nc.sync.dma_start(out=bq_sb[:], in_=bq_d[:])
        bkv_sb = const.tile([128, 1], F32, tag="bkv")
        nc.sync.dma_start(out=bkv_sb[:], in_=bkv_d[:])
        zb = const.tile([128, 1], F32, tag="zb")
        nc.vector.memset(zb[:], 0.0)
        for b in range(B):
            for sc in range(NSC):
                if (b, sc) == (0, 0):
                    continue
                nc.sync.dma_start(out=hst[b][sc][:], in_=hsr_d[b, sc])

        qT = [[None, None] for _ in range(B)]
        kvT = [None] * B
        kth = [None] * B
        # V tiles pre-created with their ones column set while the DMAs of
        # the first hs chunks are still in flight (DVE is idle then).
        v1 = [[None] * NKT for _ in range(B)]
        for b in range(B):
            for kt in range(NKT):
                v = v1p.tile([128, 65], BF, tag="v1", name=f"v1_{b}_{kt}")
                nc.vector.memset(v[:, 64:65], 1.0)
                v1[b][kt] = v

        def init_b(b):
            qT[b][0] = qtp.tile([128, S], BF, tag="qt", name=f"qT{b}_0")
            qT[b][1] = qtp.tile([128, S], BF, tag="qt", name=f"qT{b}_1")
            kvT[b] = kvp.tile([128, S], BF, tag="kv", name=f"kvT{b}")
            kth[b] = kthp.tile([128, S], BF, tag="kth", name=f"kth{b}")

        def chunk_units(b, sc):
            """Projections + V tiles + partial kth copy for hs chunk sc of
            batch b; one engine instruction per yield."""
            hs_t = hst[b][sc]
            c0 = sc * 512
            # Chain order Q0, KV, Q1: attention on heads 0/1 only needs the
            # first two, so the prologue can start scores one chain earlier.
            for part in ("q0", "kv", "q1"):
                ps = psap.tile([128, 512], F32, tag="ps")
                if part == "kv":
                    for dt_ in range(NDT):
                        nc.tensor.matmul(
                            ps[:], wkv_sb[:, dt_ * 128:(dt_ + 1) * 128],
                            hs_t[:, dt_ * 512:(dt_ + 1) * 512],
                            start=(dt_ == 0), stop=(dt_ == NDT - 1),
                        )
                        yield
                    nc.vector.tensor_scalar_add(
                        kvT[b][:, c0:c0 + 512], ps[:], bkv_sb[:])
                    yield
                else:
                    qc = 0 if part == "q0" else 1
                    for dt_ in range(NDT):
                        nc.tensor.matmul(
                            ps[:],
                            wq_sb[:, dt_ * 256 + qc * 128:
                                  dt_ * 256 + (qc + 1) * 128],
                            hs_t[:, dt_ * 512:(dt_ + 1) * 512],
                            start=(dt_ == 0), stop=(dt_ == NDT - 1),
                        )
                        yield
                    nc.vector.tensor_scalar_add(
                        qT[b][qc][:, c0:c0 + 512], ps[:], bq_sb[:, qc:qc + 1])
                    yield
            # K^T rows shifted to partitions 64:127 for odd heads; issued
            # from the Pool queue so the SP queue (hsT loads) is not blocked.
            nc.gpsimd.dma_start(out=kth[b][64:128, c0:c0 + 512],
                                in_=kvT[b][0:64, c0:c0 + 512])
            yield
            # V natural tiles [s_k 128, 64]; the ones column was pre-set.
            for kt in range(sc * 4, sc * 4 + 4):
                pst = psap.tile([128, 512], BF, tag="ps", name=f"pst{b}_{kt}")
                nc.tensor.transpose(
                    pst[:, 0:64], kvT[b][64:128, kt * 128:(kt + 1) * 128],
                    ident[64:128, 64:128],
                )
                yield
                nc.vector.tensor_copy(v1[b][kt][:, 0:64], pst[:, 0:64])
                yield

        def gen_a(b):
            init_b(b)
            for sc in range(NSC):
                yield from chunk_units(b, sc)

        # ---- phase C machinery ----
        outt = {0: [None] * (NQC * QTPC), 1: [None] * (NQC * QTPC)}
        exs = {}
        ctxs = {}

        def coff(qi):
            # qi 0..6 packed in bank 0; qi 7 at the bank-1 boundary so no
            # accumulation group straddles a PSUM bank.
            return qi * 65 if qi < 7 else 512

        def ex_stat(bkey, kt, qi):
            for ex, qi0, nqi in exs[bkey][kt]:
                if qi0 <= qi < qi0 + nqi:
                    j = qi - qi0
                    return ex[:, j * 128:(j + 1) * 128]
            raise AssertionError(f"no exp span for {bkey} kt={kt} qi={qi}")

        def pv_chunk(bkey, it, swap=False):
            # it 0..15: two passes of 8 k-tiles per q-tile qi = it//2.
            # swap=True consumes k-tiles 8..15 on the first pass (used when
            # the producing chunk emitted its exps in swapped order).
            b, qc, h = bkey
            ctx_t = ctxs[bkey]
            qi = it // 2
            base = (it % 2) * 8
            if swap:
                base = 8 - base
            for k2 in range(8):
                kt = base + k2
                nc.tensor.matmul(
                    ctx_t[:, coff(qi):coff(qi) + 65],
                    ex_stat(bkey, kt, qi),
                    v1[b][kt][:],
                    start=(it % 2 == 0 and k2 == 0),
                    stop=(it % 2 == 1 and k2 == 7),
                )

        def normalize_qi(bkey, qi):
            b, qc, h = bkey
            ctx_t = ctxs[bkey]
            qtile = qc * QTPC + qi
            if h == 0:
                outt[b][qtile] = outp.tile([128, MCOLS], F32, tag="out",
                                           name=f"outt{b}_{qtile}")
            rec = recp.tile([128, 1], F32, tag="rec")
            nc.vector.reciprocal(
                rec[:], ctx_t[:, coff(qi) + 64:coff(qi) + 65])
            nc.vector.tensor_scalar_mul(
                outt[b][qtile][:, h * 64:(h + 1) * 64],
                ctx_t[:, coff(qi):coff(qi) + 64], rec[:])
            if h == QH - 1:
                nc.sync.dma_start(
                    out=out_d[b, qtile * 128:(qtile + 1) * 128, :],
                    in_=outt[b][qtile][:])

        def begin_chunk(bkey):
            b, qc, h = bkey
            ctxs[bkey] = ctxp.tile([128, 577], F32, tag="ctx",
                                   name=f"ctx{b}_{qc}_{h}")
            exs[bkey] = []

        def score_exp(bkey, kt, qi0=0, nqi=QTPC):
            b, qc, h = bkey
            r0 = (h % 2) * 64
            kmat = kvT[b] if r0 == 0 else kth[b]
            qt = qT[b][h // 2]
            w = nqi * 128
            pss = pssp.tile([128, 1024], F32, tag="pss")
            for j in range(0, w, 512):
                jw = min(512, w - j)
                q0 = qc * 1024 + qi0 * 128 + j
                nc.tensor.matmul(
                    pss[:, j:j + jw],
                    kmat[r0:r0 + 64, kt * 128:(kt + 1) * 128],
                    qt[r0:r0 + 64, q0:q0 + jw],
                    start=True, stop=True,
                )
            ex = expp.tile([128, 1024], BF, tag="ex")
            nc.scalar.activation(ex[:, 0:w], pss[:, 0:w], Exp, bias=zb[:])
            while len(exs[bkey]) <= kt:
                exs[bkey].append([])
            exs[bkey][kt].append((ex, qi0, nqi))

        def finish_chunk(bkey):
            del exs[bkey]
            del ctxs[bkey]

        def advance(gen, n):
            if gen is None:
                return None
            for _ in range(n):
                if next(gen, StopIteration) is StopIteration:
                    return None
            return gen

        # ---- emission ----
        # Batch 0 prologue: after hs chunk 0 is projected, scores/exp for
        # (qc0, h0) start immediately on the q-tiles and k-tiles that exist,
        # widening as chunks 1-3 land (interleaved into the same iterations).
        init_b(0)
        for _ in chunk_units(0, 0):
            pass
        key00 = (0, 0, 0)
        begin_chunk(key00)
        g = chunk_units(0, 1)
        for kt in range(4):               # k 0:512 x q 0:512
            score_exp(key00, kt, 0, 4)
            g = advance(g, 15)
        if g is not None:
            for _ in g:
                pass
        g = chunk_units(0, 2)
        for kt in range(4):               # k 0:512 x q 512:1024
            score_exp(key00, kt, 4, 4)
            g = advance(g, 8)
        for kt in range(4, 8):            # k 512:1024 x q 0:1024
            score_exp(key00, kt, 0, 8)
            g = advance(g, 8)
        if g is not None:
            for _ in g:
                pass
        g = chunk_units(0, 3)
        for kt in range(8, 12):           # k 1024:1536 x q 0:1024
            score_exp(key00, kt, 0, 8)
            g = advance(g, 15)
        if g is not None:
            for _ in g:
                pass
        for kt in range(12, NKT):         # k 1536:2048 x q 0:1024
            score_exp(key00, kt, 0, 8)

        # Steady state: remaining 7 chunks of batch 0 with batch 1's
        # projections interleaved, then batch 1's chunks.
        order0 = [(0, qc, h) for qc in range(NQC) for h in range(QH)][1:]
        order1 = [(1, qc, h) for qc in range(NQC) for h in range(QH)]
        g1 = gen_a(1)
        prev = key00
        for bkey in order0:
            begin_chunk(bkey)
            for kt in range(NKT):
                score_exp(bkey, kt)
                pv_chunk(prev, kt)
                if kt % 2 == 1:
                    normalize_qi(prev, kt // 2)
                g1 = advance(g1, 2)
            finish_chunk(prev)
            prev = bkey
        # flush batch 1 projections before its attention begins
        if g1 is not None:
            for _ in g1:
                pass
        for bkey in order1:
            last = bkey == order1[-1]
            begin_chunk(bkey)
            for i, kt in enumerate(range(NKT)):
                # The final chunk emits k-tiles 8..15 first so its PV flush
                # (which consumes the late k-tiles on odd passes) is never
                # waiting on the exp backlog at the very end.
                score_exp(bkey, (kt + 8) % NKT if last else kt)
                pv_chunk(prev, kt)
                if kt % 2 == 1:
                    normalize_qi(prev, kt // 2)
            finish_chunk(prev)
            prev = bkey
        for it in range(NKT):
            pv_chunk(prev, it, swap=True)
            if it % 2 == 1:
                normalize_qi(prev, it // 2)
        finish_chunk(prev)

    nc.compile()
    return nc


def make_in_maps(hidden_states, Wq, bq, Wk, bk, Wv, bv):
    bf = mybir.dt.np(BF)
    hs = np.asarray(hidden_states, dtype=np.float32)
    Wq = np.asarray(Wq, dtype=np.float32)
    bq = np.asarray(bq, dtype=np.float32)
    Wk = np.asarray(Wk, dtype=np.float32)
    bk = np.asarray(bk, dtype=np.float32)
    Wv = np.asarray(Wv, dtype=np.float32)
    bv = np.asarray(bv, dtype=np.float32)
    sc = 1.0 / np.sqrt(np.float32(HD))
    # [b, sc, p(d in tile), t(d tile), j(s in chunk)] -> [2, 4, 128, 8192]
    hsr = np.ascontiguousarray(
        hs.reshape(B, NSC, 512, NDT, 128).transpose(0, 1, 4, 3, 2)
        .reshape(B, NSC, 128, NDT * 512).astype(bf))
    ident = np.eye(128, dtype=bf)
    in_maps = []
    for c in range(NCORES):
        qs = slice(c * MCOLS, (c + 1) * MCOLS)
        ks = slice(c * HD, (c + 1) * HD)
        wqs = (Wq[:, qs] * sc).astype(bf)
        wqr = np.ascontiguousarray(
            wqs.reshape(NDT, 128, MCOLS).transpose(1, 0, 2)
            .reshape(128, NDT * MCOLS))
        wkvs = np.concatenate([Wk[:, ks], Wv[:, ks]], axis=1).astype(bf)
        wkvr = np.ascontiguousarray(
            wkvs.reshape(NDT, 128, 128).transpose(1, 0, 2)
            .reshape(128, NDT * 128))
        bq_c = np.ascontiguousarray((bq[qs] * sc).reshape(2, 128).T)
        in_maps.append({
            "hsr": hsr,
            "wqr": wqr,
            "wkvr": wkvr,
            "bq": bq_c,
            "bkv": np.concatenate([bk[ks], bv[ks]]).reshape(128, 1),
            "ident": ident,
        })
    return in_maps


_NC_CACHE = {}


def get_nc():
    if "nc" not in _NC_CACHE:
        _NC_CACHE["nc"] = build_nc()
    return _NC_CACHE["nc"]


def kernel(hidden_states, Wq, bq, Wk, bk, Wv, bv):
    nc = get_nc()
    in_maps = make_in_maps(hidden_states, Wq, bq, Wk, bk, Wv, bv)
    res = run_bass_kernel_spmd(nc, in_maps, list(range(NCORES)))
    outs = [np.asarray(r["out"], dtype=np.float32) for r in res.results]
    return np.concatenate(outs, axis=-1)


# revision 24
# speedup vs baseline: 1.8144x; 1.0065x over previous
"""Trainium2 Bass kernel for GroupedQueryAttention (v3, bf16, early-start).

Sharding: 8 cores; core c owns KV head g=c and Q heads 4c..4c+3, both batch
elements. Each core computes its [2, 2048, 256] output slice; host concats.

Design:
  * hs is pre-transposed AND pre-cast to bf16 on the host (hsr layout:
    [b, s-chunk, d-in-tile(128), d-tile(16) x s(512)]), removing all 512
    PE transposes and their DVE drains from the device program.
  * All PE operands are bf16 (1.0 cycles/row at any ap size), PSUM fp32.
  * PV uses expS^T tiles as the STATIONARY and natural [V|1] as the moving
    ([128 k, 65]): output is ctx in natural [q, d] orientation with the
    softmax denominator in column 64 -> no output transposes, and PV costs
    65 rows per (k-tile, q-tile) instead of 512 per (k-tile, 512q).
  * exp runs on ACT in [128, 1024] chunks (double-buffered PSUM); ACT is
    the global bottleneck (~267us busy), so the schedule keeps it saturated:
    - early start: attention chunk (qc0,h0) k-tiles 0..7 begin right after
      hs-chunks 0,1 are projected; hs-chunks 2,3 interleave into those
      iterations, so the first exp fires ~35us earlier than a sequential
      phase-A/phase-C split;
    - batch 1's entire projection phase is interleaved into batch 0's
      attention loop (~2 PE instructions per k-tile iteration);
    - PV for chunk (qc,h) is software-pipelined into the scores/exp loop of
      the next chunk so ctx accumulation groups stay sequential per PSUM
      zero region (hardware allows one open group per 2KB bank).
"""

import sys
from contextlib import ExitStack

import numpy as np

sys.path.insert(0, "/opt/trn_rl_repo")

import concourse.bass as bass  # noqa: E402
import concourse.bacc as bacc  # noqa: E402
import concourse.tile as tile  # noqa: E402
from concourse import mybir  # noqa: E402
from concourse.bass_utils import run_bass_kernel_spmd  # noqa: E402

B = 2
S = 2048
D = 2048
HD = 64
NCORES = 8
QH = 4           # q heads per core
MCOLS = QH * HD  # 256 output cols per core

BF = mybir.dt.bfloat16
F32 = mybir.dt.float32
Exp = mybir.ActivationFunctionType.Exp

NDT = 16         # d tiles of 128
NSC = 4          # s chunks of 512 per batch
NKT = 16         # s_k tiles of 128
NQC = 2          # q chunks of 1024 per batch
QTPC = 8         # q tiles of 128 per q chunk


def build_nc():
    nc = bacc.Bacc("TRN2", target_bir_lowering=False, debug=False)

    hsr_d = nc.dram_tensor("hsr", [B, NSC, 128, NDT * 512], BF,
                           kind="ExternalInput")
    wqr_d = nc.dram_tensor("wqr", [128, NDT * 256], BF, kind="ExternalInput")
    wkvr_d = nc.dram_tensor("wkvr", [128, NDT * 128], BF, kind="ExternalInput")
    bq_d = nc.dram_tensor("bq", [128, 2], F32, kind="ExternalInput")
    bkv_d = nc.dram_tensor("bkv", [128, 1], F32, kind="ExternalInput")
    id_d = nc.dram_tensor("ident", [128, 128], BF, kind="ExternalInput")
    out_d = nc.dram_tensor("out", [B, S, MCOLS], F32, kind="ExternalOutput")

    with tile.TileContext(nc) as tc, ExitStack() as ctx:
        const = ctx.enter_context(tc.tile_pool(name="const", bufs=1))
        wqp = ctx.enter_context(tc.tile_pool(name="wqp", bufs=1))
        hstp = ctx.enter_context(tc.tile_pool(name="hstp", bufs=4))
        qtp = ctx.enter_context(tc.tile_pool(name="qtp", bufs=4))
        kvp = ctx.enter_context(tc.tile_pool(name="kvp", bufs=2))
        kthp = ctx.enter_context(tc.tile_pool(name="kthp", bufs=2))
        v1p = ctx.enter_context(tc.tile_pool(name="v1p", bufs=2 * NKT))
        expp = ctx.enter_context(tc.tile_pool(name="expp", bufs=38))
        recp = ctx.enter_context(tc.tile_pool(name="recp", bufs=4))
        outp = ctx.enter_context(tc.tile_pool(name="outp", bufs=16))
        psap = ctx.enter_context(tc.tile_pool(name="psap", bufs=2, space="PSUM"))
        pssp = ctx.enter_context(tc.tile_pool(name="pssp", bufs=2, space="PSUM"))
        ctxp = ctx.enter_context(tc.tile_pool(name="ctxp", bufs=1, space="PSUM"))

        # DMA order is the cold-start critical path: Wq first, then hs chunk
        # 0, so the first projection chain can begin ~9us in; everything else
        # follows.
        wq_sb = wqp.tile([128, NDT * 256], BF, tag="wq")
        nc.sync.dma_start(out=wq_sb[:], in_=wqr_d[:])
        hst = [[None] * NSC for _ in range(B)]
        for b in range(B):
            for sc in range(NSC):
                hst[b][sc] = hstp.tile([128, NDT * 512], BF, tag="hst",
                                       name=f"hst{b}_{sc}")
        nc.sync.dma_start(out=hst[0][0][:], in_=hsr_d[0, 0])
        wkv_sb = wqp.tile([128, NDT * 128], BF, tag="wkv")
        nc.sync.dma_start(out=wkv_sb[:], in_=wkvr_d[:])
        ident = const.tile([128, 128], BF, tag="ident")
        nc.sync.dma_start(out=ident[:], in_=id_d[:])
        bq_sb = const.tile([128, 2], F32, tag="bq")
        nc.sync.dma_start(out=bq_sb[:], in_=bq_d[:])
        bkv_sb = const.tile([128, 1], F32, tag="bkv")
        nc.sync.dma_start(out=bkv_sb[:], in_=bkv_d[:])
        zb = const.tile([128, 1], F32, tag="zb")
        nc.vector.memset(zb[:], 0.0)
        for b in range(B):
            for sc in range(NSC):
                if (b, sc) == (0, 0):
                    continue
                nc.sync.dma_start(out=hst[b][sc][:], in_=hsr_d[b, sc])

        qT = [[None, None] for _ in range(B)]
        kvT = [None] * B
        kth = [None] * B
        # V tiles pre-created with their ones column set while the DMAs of
        # the first hs chunks are still in flight (DVE is idle then).
        v1 = [[None] * NKT for _ in range(B)]
        for b in range(B):
            for kt in range(NKT):
                v = v1p.tile([128, 65], BF, tag="v1", name=f"v1_{b}_{kt}")
                nc.vector.memset(v[:, 64:65], 1.0)
                v1[b][kt] = v

        def init_b(b):
            qT[b][0] = qtp.tile([128, S], BF, tag="qt", name=f"qT{b}_0")
            qT[b][1] = qtp.tile([128, S], BF, tag="qt", name=f"qT{b}_1")
            kvT[b] = kvp.tile([128, S], BF, tag="kv", name=f"kvT{b}")
            kth[b] = kthp.tile([128, S], BF, tag="kth", name=f"kth{b}")

        def q_chain(b, sc, qc):
            hs_t = hst[b][sc]
            c0 = sc * 512
            ps = psap.tile([128, 512], F32, tag="ps")
            for dt_ in range(NDT):
                nc.tensor.matmul(
                    ps[:],
                    wq_sb[:, dt_ * 256 + qc * 128:dt_ * 256 + (qc + 1) * 128],
                    hs_t[:, dt_ * 512:(dt_ + 1) * 512],
                    start=(dt_ == 0), stop=(dt_ == NDT - 1),
                )
                yield
            nc.vector.tensor_scalar_add(
                qT[b][qc][:, c0:c0 + 512], ps[:], bq_sb[:, qc:qc + 1])
            yield

        def chunk_q0kv(b, sc):
            """Q heads 0/1 + KV projections, kth copy and V tiles for hs
            chunk sc.  Everything attention on heads 0/1 needs; the heads
            2/3 projection (q_chain qc=1) can run much later."""
            hs_t = hst[b][sc]
            c0 = sc * 512
            yield from q_chain(b, sc, 0)
            ps = psap.tile([128, 512], F32, tag="ps")
            for dt_ in range(NDT):
                nc.tensor.matmul(
                    ps[:], wkv_sb[:, dt_ * 128:(dt_ + 1) * 128],
                    hs_t[:, dt_ * 512:(dt_ + 1) * 512],
                    start=(dt_ == 0), stop=(dt_ == NDT - 1),
                )
                yield
            nc.vector.tensor_scalar_add(
                kvT[b][:, c0:c0 + 512], ps[:], bkv_sb[:])
            yield
            # K^T rows shifted to partitions 64:127 for odd heads; issued
            # from the Pool queue so the SP queue (hsT loads) is not blocked.
            nc.gpsimd.dma_start(out=kth[b][64:128, c0:c0 + 512],
                                in_=kvT[b][0:64, c0:c0 + 512])
            yield
            # V natural tiles [s_k 128, 64]; the ones column was pre-set.
            for kt in range(sc * 4, sc * 4 + 4):
                pst = psap.tile([128, 512], BF, tag="ps", name=f"pst{b}_{kt}")
                nc.tensor.transpose(
                    pst[:, 0:64], kvT[b][64:128, kt * 128:(kt + 1) * 128],
                    ident[64:128, 64:128],
                )
                yield
                nc.vector.tensor_copy(v1[b][kt][:, 0:64], pst[:, 0:64])
                yield

        def gen_a(b):
            init_b(b)
            for sc in range(NSC):
                yield from chunk_q0kv(b, sc)
                yield from q_chain(b, sc, 1)

        # ---- phase C machinery ----
        outt = {0: [None] * (NQC * QTPC), 1: [None] * (NQC * QTPC)}
        exs = {}
        ctxs = {}

        def coff(qi):
            # qi 0..6 packed in bank 0; qi 7 at the bank-1 boundary so no
            # accumulation group straddles a PSUM bank.
            return qi * 65 if qi < 7 else 512

        def ex_stat(bkey, kt, qi):
            for ex, qi0, nqi in exs[bkey][kt]:
                if qi0 <= qi < qi0 + nqi:
                    j = qi - qi0
                    return ex[:, j * 128:(j + 1) * 128]
            raise AssertionError(f"no exp span for {bkey} kt={kt} qi={qi}")

        def pv_chunk(bkey, it, swap=False):
            # it 0..15: two passes of 8 k-tiles per q-tile qi = it//2.
            # swap=True consumes k-tiles 8..15 on the first pass (used when
            # the producing chunk emitted its exps in swapped order).
            b, qc, h = bkey
            ctx_t = ctxs[bkey]
            qi = it // 2
            base = (it % 2) * 8
            if swap:
                base = 8 - base
            for k2 in range(8):
                kt = base + k2
                nc.tensor.matmul(
                    ctx_t[:, coff(qi):coff(qi) + 65],
                    ex_stat(bkey, kt, qi),
                    v1[b][kt][:],
                    start=(it % 2 == 0 and k2 == 0),
                    stop=(it % 2 == 1 and k2 == 7),
                )

        def normalize_qi(bkey, qi):
            b, qc, h = bkey
            ctx_t = ctxs[bkey]
            qtile = qc * QTPC + qi
            if h == 0:
                outt[b][qtile] = outp.tile([128, MCOLS], F32, tag="out",
                                           name=f"outt{b}_{qtile}")
            rec = recp.tile([128, 1], F32, tag="rec")
            nc.vector.reciprocal(
                rec[:], ctx_t[:, coff(qi) + 64:coff(qi) + 65])
            nc.vector.tensor_scalar_mul(
                outt[b][qtile][:, h * 64:(h + 1) * 64],
                ctx_t[:, coff(qi):coff(qi) + 64], rec[:])
            if h == QH - 1:
                nc.sync.dma_start(
                    out=out_d[b, qtile * 128:(qtile + 1) * 128, :],
                    in_=outt[b][qtile][:])

        def begin_chunk(bkey):
            b, qc, h = bkey
            ctxs[bkey] = ctxp.tile([128, 577], F32, tag="ctx",
                                   name=f"ctx{b}_{qc}_{h}")
            exs[bkey] = []

        def score_exp(bkey, kt, qi0=0, nqi=QTPC):
            b, qc, h = bkey
            r0 = (h % 2) * 64
            kmat = kvT[b] if r0 == 0 else kth[b]
            qt = qT[b][h // 2]
            w = nqi * 128
            pss = pssp.tile([128, 1024], F32, tag="pss")
            for j in range(0, w, 512):
                jw = min(512, w - j)
                q0 = qc * 1024 + qi0 * 128 + j
                nc.tensor.matmul(
                    pss[:, j:j + jw],
                    kmat[r0:r0 + 64, kt * 128:(kt + 1) * 128],
                    qt[r0:r0 + 64, q0:q0 + jw],
                    start=True, stop=True,
                )
            ex = expp.tile([128, 1024], BF, tag="ex")
            nc.scalar.activation(ex[:, 0:w], pss[:, 0:w], Exp, bias=zb[:])
            while len(exs[bkey]) <= kt:
                exs[bkey].append([])
            exs[bkey][kt].append((ex, qi0, nqi))

        def finish_chunk(bkey):
            del exs[bkey]
            del ctxs[bkey]

        def advance(gen, n):
            if gen is None:
                return None
            for _ in range(n):
                if next(gen, StopIteration) is StopIteration:
                    return None
            return gen

        # ---- emission ----
        # Batch 0 prologue: after hs chunk 0's Q0/KV projections, scores/exp
        # for (qc0, h0) start immediately on the q/k tiles that exist,
        # widening as later chunks land.  Each 4-iteration phase interleaves
        # exactly the projection work the NEXT phase needs, so batch 0's
        # bulk is spread across the whole prologue instead of serializing
        # in front of it.
        init_b(0)
        for _ in chunk_q0kv(0, 0):
            pass
        key00 = (0, 0, 0)
        begin_chunk(key00)
        phases = [
            (range(4), 0, 4, chunk_q0kv(0, 1), 11),   # k 0:512 x q 0:512
            (range(4), 4, 4, chunk_q0kv(0, 2), 11),   # k 0:512 x q 512:1024
            (range(4, 8), 0, 8, chunk_q0kv(0, 3), 11),
            (range(8, 12), 0, 8, q_chain(0, 0, 1), 5),
            (range(12, 16), 0, 8, q_chain(0, 1, 1), 5),
        ]
        for kts, qi0, nqi, g, rate in phases:
            for kt in kts:
                score_exp(key00, kt, qi0, nqi)
                g = advance(g, rate)
            if g is not None:
                for _ in g:
                    pass

        # Steady state: remaining 7 chunks of batch 0 with the deferred
        # batch-0 head-2/3 projections and batch 1's full projection phase
        # interleaved at an adaptive rate, then batch 1's chunks.
        order0 = [(0, qc, h) for qc in range(NQC) for h in range(QH)][1:]
        order1 = [(1, qc, h) for qc in range(NQC) for h in range(QH)]

        def steady_units():
            yield from q_chain(0, 2, 1)
            yield from q_chain(0, 3, 1)
            yield from gen_a(1)

        g1 = steady_units()
        prev = key00
        for bkey in order0:
            begin_chunk(bkey)
            for kt in range(NKT):
                score_exp(bkey, kt)
                pv_chunk(prev, kt)
                if kt % 2 == 1:
                    normalize_qi(prev, kt // 2)
                g1 = advance(g1, 2)
            finish_chunk(prev)
            prev = bkey
        # flush batch 1 projections before its attention begins
        if g1 is not None:
            for _ in g1:
                pass
        for bkey in order1:
            last = bkey == order1[-1]
            begin_chunk(bkey)
            for i, kt in enumerate(range(NKT)):
                # The final chunk emits k-tiles 8..15 first so its PV flush
                # (which consumes the late k-tiles on odd passes) is never
                # waiting on the exp backlog at the very end.
                score_exp(bkey, (kt + 8) % NKT if last else kt)
                pv_chunk(prev, kt)
                if kt % 2 == 1:
                    normalize_qi(prev, kt // 2)
            finish_chunk(prev)
            prev = bkey
        for it in range(NKT):
            pv_chunk(prev, it, swap=True)
            if it % 2 == 1:
                normalize_qi(prev, it // 2)
        finish_chunk(prev)

    nc.compile()
    return nc


def make_in_maps(hidden_states, Wq, bq, Wk, bk, Wv, bv):
    bf = mybir.dt.np(BF)
    hs = np.asarray(hidden_states, dtype=np.float32)
    Wq = np.asarray(Wq, dtype=np.float32)
    bq = np.asarray(bq, dtype=np.float32)
    Wk = np.asarray(Wk, dtype=np.float32)
    bk = np.asarray(bk, dtype=np.float32)
    Wv = np.asarray(Wv, dtype=np.float32)
    bv = np.asarray(bv, dtype=np.float32)
    sc = 1.0 / np.sqrt(np.float32(HD))
    # [b, sc, p(d in tile), t(d tile), j(s in chunk)] -> [2, 4, 128, 8192]
    hsr = np.ascontiguousarray(
        hs.reshape(B, NSC, 512, NDT, 128).transpose(0, 1, 4, 3, 2)
        .reshape(B, NSC, 128, NDT * 512).astype(bf))
    ident = np.eye(128, dtype=bf)
    in_maps = []
    for c in range(NCORES):
        qs = slice(c * MCOLS, (c + 1) * MCOLS)
        ks = slice(c * HD, (c + 1) * HD)
        wqs = (Wq[:, qs] * sc).astype(bf)
        wqr = np.ascontiguousarray(
            wqs.reshape(NDT, 128, MCOLS).transpose(1, 0, 2)
            .reshape(128, NDT * MCOLS))
        wkvs = np.concatenate([Wk[:, ks], Wv[:, ks]], axis=1).astype(bf)
        wkvr = np.ascontiguousarray(
            wkvs.reshape(NDT, 128, 128).transpose(1, 0, 2)
            .reshape(128, NDT * 128))
        bq_c = np.ascontiguousarray((bq[qs] * sc).reshape(2, 128).T)
        in_maps.append({
            "hsr": hsr,
            "wqr": wqr,
            "wkvr": wkvr,
            "bq": bq_c,
            "bkv": np.concatenate([bk[ks], bv[ks]]).reshape(128, 1),
            "ident": ident,
        })
    return in_maps


_NC_CACHE = {}


def get_nc():
    if "nc" not in _NC_CACHE:
        _NC_CACHE["nc"] = build_nc()
    return _NC_CACHE["nc"]


def kernel(hidden_states, Wq, bq, Wk, bk, Wv, bv):
    nc = get_nc()
    in_maps = make_in_maps(hidden_states, Wq, bq, Wk, bk, Wv, bv)
    res = run_bass_kernel_spmd(nc, in_maps, list(range(NCORES)))
    outs = [np.asarray(r["out"], dtype=np.float32) for r in res.results]
    return np.concatenate(outs, axis=-1)


# revision 25
# speedup vs baseline: 1.8739x; 1.0328x over previous
"""Trainium2 Bass kernel for GroupedQueryAttention (v3, bf16, early-start).

Sharding: 8 cores; core c owns KV head g=c and Q heads 4c..4c+3, both batch
elements. Each core computes its [2, 2048, 256] output slice; host concats.

Design:
  * hs is pre-transposed AND pre-cast to bf16 on the host (hsr layout:
    [b, s-chunk, d-in-tile(128), d-tile(16) x s(512)]), removing all 512
    PE transposes and their DVE drains from the device program.
  * All PE operands are bf16 (1.0 cycles/row at any ap size), PSUM fp32.
  * PV uses expS^T tiles as the STATIONARY and natural [V|1] as the moving
    ([128 k, 65]): output is ctx in natural [q, d] orientation with the
    softmax denominator in column 64 -> no output transposes, and PV costs
    65 rows per (k-tile, q-tile) instead of 512 per (k-tile, 512q).
  * exp runs on ACT in [128, 1024] chunks (double-buffered PSUM); ACT is
    the global bottleneck (~267us busy), so the schedule keeps it saturated:
    - early start: attention chunk (qc0,h0) k-tiles 0..7 begin right after
      hs-chunks 0,1 are projected; hs-chunks 2,3 interleave into those
      iterations, so the first exp fires ~35us earlier than a sequential
      phase-A/phase-C split;
    - batch 1's entire projection phase is interleaved into batch 0's
      attention loop (~2 PE instructions per k-tile iteration);
    - PV for chunk (qc,h) is software-pipelined into the scores/exp loop of
      the next chunk so ctx accumulation groups stay sequential per PSUM
      zero region (hardware allows one open group per 2KB bank).
"""

import sys
from contextlib import ExitStack

import numpy as np

sys.path.insert(0, "/opt/trn_rl_repo")

import concourse.bass as bass  # noqa: E402
import concourse.bacc as bacc  # noqa: E402
import concourse.tile as tile  # noqa: E402
from concourse import mybir  # noqa: E402
from concourse.bass_utils import run_bass_kernel_spmd  # noqa: E402

B = 2
S = 2048
D = 2048
HD = 64
NCORES = 8
QH = 4           # q heads per core
MCOLS = QH * HD  # 256 output cols per core

BF = mybir.dt.bfloat16
F32 = mybir.dt.float32
Exp = mybir.ActivationFunctionType.Exp

NDT = 16         # d tiles of 128
NSC = 4          # s chunks of 512 per batch
NKT = 16         # s_k tiles of 128
NQC = 2          # q chunks of 1024 per batch
QTPC = 8         # q tiles of 128 per q chunk


def build_nc():
    nc = bacc.Bacc("TRN2", target_bir_lowering=False, debug=False)

    hsr_d = nc.dram_tensor("hsr", [B, NSC, 128, NDT * 512], BF,
                           kind="ExternalInput")
    wqr_d = nc.dram_tensor("wqr", [128, NDT * 256], BF, kind="ExternalInput")
    wkvr_d = nc.dram_tensor("wkvr", [128, NDT * 128], BF, kind="ExternalInput")
    bq_d = nc.dram_tensor("bq", [128, 2], F32, kind="ExternalInput")
    bkv_d = nc.dram_tensor("bkv", [128, 1], F32, kind="ExternalInput")
    id_d = nc.dram_tensor("ident", [128, 128], BF, kind="ExternalInput")
    out_d = nc.dram_tensor("out", [B, S, MCOLS], F32, kind="ExternalOutput")

    with tile.TileContext(nc) as tc, ExitStack() as ctx:
        const = ctx.enter_context(tc.tile_pool(name="const", bufs=1))
        wqp = ctx.enter_context(tc.tile_pool(name="wqp", bufs=1))
        hstp = ctx.enter_context(tc.tile_pool(name="hstp", bufs=4))
        qtp = ctx.enter_context(tc.tile_pool(name="qtp", bufs=4))
        kvp = ctx.enter_context(tc.tile_pool(name="kvp", bufs=2))
        kthp = ctx.enter_context(tc.tile_pool(name="kthp", bufs=2))
        v1p = ctx.enter_context(tc.tile_pool(name="v1p", bufs=2 * NKT))
        expp = ctx.enter_context(tc.tile_pool(name="expp", bufs=38))
        recp = ctx.enter_context(tc.tile_pool(name="recp", bufs=4))
        outp = ctx.enter_context(tc.tile_pool(name="outp", bufs=16))
        psap = ctx.enter_context(tc.tile_pool(name="psap", bufs=2, space="PSUM"))
        pssp = ctx.enter_context(tc.tile_pool(name="pssp", bufs=2, space="PSUM"))
        ctxp = ctx.enter_context(tc.tile_pool(name="ctxp", bufs=1, space="PSUM"))

        # DMA order is the cold-start critical path: Wq first, then hs chunk
        # 0, so the first projection chain can begin ~9us in; everything else
        # follows.
        wq_sb = wqp.tile([128, NDT * 256], BF, tag="wq")
        nc.sync.dma_start(out=wq_sb[:], in_=wqr_d[:])
        hst = [[None] * NSC for _ in range(B)]
        for b in range(B):
            for sc in range(NSC):
                hst[b][sc] = hstp.tile([128, NDT * 512], BF, tag="hst",
                                       name=f"hst{b}_{sc}")
        nc.sync.dma_start(out=hst[0][0][:], in_=hsr_d[0, 0])
        wkv_sb = wqp.tile([128, NDT * 128], BF, tag="wkv")
        nc.sync.dma_start(out=wkv_sb[:], in_=wkvr_d[:])
        ident = const.tile([128, 128], BF, tag="ident")
        nc.sync.dma_start(out=ident[:], in_=id_d[:])
        bq_sb = const.tile([128, 2], F32, tag="bq")
        nc.sync.dma_start(out=bq_sb[:], in_=bq_d[:])
        bkv_sb = const.tile([128, 1], F32, tag="bkv")
        nc.sync.dma_start(out=bkv_sb[:], in_=bkv_d[:])
        zb = const.tile([128, 1], F32, tag="zb")
        nc.vector.memset(zb[:], 0.0)

        # PE p-state warmup: the tensor engine only reaches full clock after
        # ~3us of continuous work, and the first real chain otherwise pays
        # the slow ramp right when ACT is starved for scores.  Burn the
        # initial weight/hs DMA wait (~11us) on dummy matmuls over a memset
        # scratch tile so the ramp completes before real work arrives.
        scr = const.tile([128, 512], BF, tag="scr")
        nc.vector.memset(scr[:], 0.0)
        for _ in range(40):
            ps = psap.tile([128, 512], F32, tag="ps")
            nc.tensor.matmul(ps[:], scr[:, 0:128], scr[:], start=True,
                             stop=True)
        for b in range(B):
            for sc in range(NSC):
                if (b, sc) == (0, 0):
                    continue
                nc.sync.dma_start(out=hst[b][sc][:], in_=hsr_d[b, sc])

        qT = [[None, None] for _ in range(B)]
        kvT = [None] * B
        kth = [None] * B
        # V tiles pre-created with their ones column set while the DMAs of
        # the first hs chunks are still in flight (DVE is idle then).
        v1 = [[None] * NKT for _ in range(B)]
        for b in range(B):
            for kt in range(NKT):
                v = v1p.tile([128, 65], BF, tag="v1", name=f"v1_{b}_{kt}")
                nc.vector.memset(v[:, 64:65], 1.0)
                v1[b][kt] = v

        def init_b(b):
            qT[b][0] = qtp.tile([128, S], BF, tag="qt", name=f"qT{b}_0")
            qT[b][1] = qtp.tile([128, S], BF, tag="qt", name=f"qT{b}_1")
            kvT[b] = kvp.tile([128, S], BF, tag="kv", name=f"kvT{b}")
            kth[b] = kthp.tile([128, S], BF, tag="kth", name=f"kth{b}")

        def q_chain(b, sc, qc):
            hs_t = hst[b][sc]
            c0 = sc * 512
            ps = psap.tile([128, 512], F32, tag="ps")
            for dt_ in range(NDT):
                nc.tensor.matmul(
                    ps[:],
                    wq_sb[:, dt_ * 256 + qc * 128:dt_ * 256 + (qc + 1) * 128],
                    hs_t[:, dt_ * 512:(dt_ + 1) * 512],
                    start=(dt_ == 0), stop=(dt_ == NDT - 1),
                )
                yield
            nc.vector.tensor_scalar_add(
                qT[b][qc][:, c0:c0 + 512], ps[:], bq_sb[:, qc:qc + 1])
            yield

        def chunk_q0kv(b, sc):
            """Q heads 0/1 + KV projections, kth copy and V tiles for hs
            chunk sc.  Everything attention on heads 0/1 needs; the heads
            2/3 projection (q_chain qc=1) can run much later."""
            hs_t = hst[b][sc]
            c0 = sc * 512
            yield from q_chain(b, sc, 0)
            ps = psap.tile([128, 512], F32, tag="ps")
            for dt_ in range(NDT):
                nc.tensor.matmul(
                    ps[:], wkv_sb[:, dt_ * 128:(dt_ + 1) * 128],
                    hs_t[:, dt_ * 512:(dt_ + 1) * 512],
                    start=(dt_ == 0), stop=(dt_ == NDT - 1),
                )
                yield
            nc.vector.tensor_scalar_add(
                kvT[b][:, c0:c0 + 512], ps[:], bkv_sb[:])
            yield
            # K^T rows shifted to partitions 64:127 for odd heads; issued
            # from the Pool queue so the SP queue (hsT loads) is not blocked.
            nc.gpsimd.dma_start(out=kth[b][64:128, c0:c0 + 512],
                                in_=kvT[b][0:64, c0:c0 + 512])
            yield
            # V natural tiles [s_k 128, 64]; the ones column was pre-set.
            for kt in range(sc * 4, sc * 4 + 4):
                pst = psap.tile([128, 512], BF, tag="ps", name=f"pst{b}_{kt}")
                nc.tensor.transpose(
                    pst[:, 0:64], kvT[b][64:128, kt * 128:(kt + 1) * 128],
                    ident[64:128, 64:128],
                )
                yield
                nc.vector.tensor_copy(v1[b][kt][:, 0:64], pst[:, 0:64])
                yield

        def gen_a(b):
            init_b(b)
            for sc in range(NSC):
                yield from chunk_q0kv(b, sc)
                yield from q_chain(b, sc, 1)

        # ---- phase C machinery ----
        outt = {0: [None] * (NQC * QTPC), 1: [None] * (NQC * QTPC)}
        exs = {}
        ctxs = {}

        def coff(qi):
            # qi 0..6 packed in bank 0; qi 7 at the bank-1 boundary so no
            # accumulation group straddles a PSUM bank.
            return qi * 65 if qi < 7 else 512

        def ex_stat(bkey, kt, qi):
            for ex, qi0, nqi in exs[bkey][kt]:
                if qi0 <= qi < qi0 + nqi:
                    j = qi - qi0
                    return ex[:, j * 128:(j + 1) * 128]
            raise AssertionError(f"no exp span for {bkey} kt={kt} qi={qi}")

        def pv_chunk(bkey, it, swap=False):
            # it 0..15: two passes of 8 k-tiles per q-tile qi = it//2.
            # swap=True consumes k-tiles 8..15 on the first pass (used when
            # the producing chunk emitted its exps in swapped order).
            b, qc, h = bkey
            ctx_t = ctxs[bkey]
            qi = it // 2
            base = (it % 2) * 8
            if swap:
                base = 8 - base
            for k2 in range(8):
                kt = base + k2
                nc.tensor.matmul(
                    ctx_t[:, coff(qi):coff(qi) + 65],
                    ex_stat(bkey, kt, qi),
                    v1[b][kt][:],
                    start=(it % 2 == 0 and k2 == 0),
                    stop=(it % 2 == 1 and k2 == 7),
                )

        def normalize_qi(bkey, qi):
            b, qc, h = bkey
            ctx_t = ctxs[bkey]
            qtile = qc * QTPC + qi
            if h == 0:
                outt[b][qtile] = outp.tile([128, MCOLS], F32, tag="out",
                                           name=f"outt{b}_{qtile}")
            rec = recp.tile([128, 1], F32, tag="rec")
            nc.vector.reciprocal(
                rec[:], ctx_t[:, coff(qi) + 64:coff(qi) + 65])
            nc.vector.tensor_scalar_mul(
                outt[b][qtile][:, h * 64:(h + 1) * 64],
                ctx_t[:, coff(qi):coff(qi) + 64], rec[:])
            if h == QH - 1:
                nc.sync.dma_start(
                    out=out_d[b, qtile * 128:(qtile + 1) * 128, :],
                    in_=outt[b][qtile][:])

        def begin_chunk(bkey):
            b, qc, h = bkey
            ctxs[bkey] = ctxp.tile([128, 577], F32, tag="ctx",
                                   name=f"ctx{b}_{qc}_{h}")
            exs[bkey] = []

        def score_exp(bkey, kt, qi0=0, nqi=QTPC):
            b, qc, h = bkey
            r0 = (h % 2) * 64
            kmat = kvT[b] if r0 == 0 else kth[b]
            qt = qT[b][h // 2]
            w = nqi * 128
            pss = pssp.tile([128, 1024], F32, tag="pss")
            for j in range(0, w, 512):
                jw = min(512, w - j)
                q0 = qc * 1024 + qi0 * 128 + j
                nc.tensor.matmul(
                    pss[:, j:j + jw],
                    kmat[r0:r0 + 64, kt * 128:(kt + 1) * 128],
                    qt[r0:r0 + 64, q0:q0 + jw],
                    start=True, stop=True,
                )
            ex = expp.tile([128, 1024], BF, tag="ex")
            nc.scalar.activation(ex[:, 0:w], pss[:, 0:w], Exp, bias=zb[:])
            while len(exs[bkey]) <= kt:
                exs[bkey].append([])
            exs[bkey][kt].append((ex, qi0, nqi))

        def finish_chunk(bkey):
            del exs[bkey]
            del ctxs[bkey]

        def advance(gen, n):
            if gen is None:
                return None
            for _ in range(n):
                if next(gen, StopIteration) is StopIteration:
                    return None
            return gen

        # ---- emission ----
        # Batch 0 prologue: after hs chunk 0's Q0/KV projections, scores/exp
        # for (qc0, h0) start immediately on the q/k tiles that exist,
        # widening as later chunks land.  Each 4-iteration phase interleaves
        # exactly the projection work the NEXT phase needs, so batch 0's
        # bulk is spread across the whole prologue instead of serializing
        # in front of it.
        init_b(0)
        for _ in chunk_q0kv(0, 0):
            pass
        key00 = (0, 0, 0)
        begin_chunk(key00)
        phases = [
            (range(4), 0, 4, chunk_q0kv(0, 1), 11),   # k 0:512 x q 0:512
            (range(4), 4, 4, chunk_q0kv(0, 2), 11),   # k 0:512 x q 512:1024
            (range(4, 8), 0, 8, chunk_q0kv(0, 3), 11),
            (range(8, 12), 0, 8, q_chain(0, 0, 1), 5),
            (range(12, 16), 0, 8, q_chain(0, 1, 1), 5),
        ]
        for kts, qi0, nqi, g, rate in phases:
            for kt in kts:
                score_exp(key00, kt, qi0, nqi)
                g = advance(g, rate)
            if g is not None:
                for _ in g:
                    pass

        # Steady state: remaining 7 chunks of batch 0 with the deferred
        # batch-0 head-2/3 projections and batch 1's full projection phase
        # interleaved at an adaptive rate, then batch 1's chunks.
        order0 = [(0, qc, h) for qc in range(NQC) for h in range(QH)][1:]
        order1 = [(1, qc, h) for qc in range(NQC) for h in range(QH)]

        def steady_units():
            yield from q_chain(0, 2, 1)
            yield from q_chain(0, 3, 1)
            yield from gen_a(1)

        g1 = steady_units()
        prev = key00
        for bkey in order0:
            begin_chunk(bkey)
            for kt in range(NKT):
                score_exp(bkey, kt)
                pv_chunk(prev, kt)
                if kt % 2 == 1:
                    normalize_qi(prev, kt // 2)
                g1 = advance(g1, 2)
            finish_chunk(prev)
            prev = bkey
        # flush batch 1 projections before its attention begins
        if g1 is not None:
            for _ in g1:
                pass
        for bkey in order1:
            last = bkey == order1[-1]
            begin_chunk(bkey)
            for i, kt in enumerate(range(NKT)):
                # The final chunk emits k-tiles 8..15 first so its PV flush
                # (which consumes the late k-tiles on odd passes) is never
                # waiting on the exp backlog at the very end.
                score_exp(bkey, (kt + 8) % NKT if last else kt)
                pv_chunk(prev, kt)
                if kt % 2 == 1:
                    normalize_qi(prev, kt // 2)
            finish_chunk(prev)
            prev = bkey
        for it in range(NKT):
            pv_chunk(prev, it, swap=True)
            if it % 2 == 1:
                normalize_qi(prev, it // 2)
        finish_chunk(prev)

    nc.compile()
    return nc


def make_in_maps(hidden_states, Wq, bq, Wk, bk, Wv, bv):
    bf = mybir.dt.np(BF)
    hs = np.asarray(hidden_states, dtype=np.float32)
    Wq = np.asarray(Wq, dtype=np.float32)
    bq = np.asarray(bq, dtype=np.float32)
    Wk = np.asarray(Wk, dtype=np.float32)
    bk = np.asarray(bk, dtype=np.float32)
    Wv = np.asarray(Wv, dtype=np.float32)
    bv = np.asarray(bv, dtype=np.float32)
    sc = 1.0 / np.sqrt(np.float32(HD))
    # [b, sc, p(d in tile), t(d tile), j(s in chunk)] -> [2, 4, 128, 8192]
    hsr = np.ascontiguousarray(
        hs.reshape(B, NSC, 512, NDT, 128).transpose(0, 1, 4, 3, 2)
        .reshape(B, NSC, 128, NDT * 512).astype(bf))
    ident = np.eye(128, dtype=bf)
    in_maps = []
    for c in range(NCORES):
        qs = slice(c * MCOLS, (c + 1) * MCOLS)
        ks = slice(c * HD, (c + 1) * HD)
        wqs = (Wq[:, qs] * sc).astype(bf)
        wqr = np.ascontiguousarray(
            wqs.reshape(NDT, 128, MCOLS).transpose(1, 0, 2)
            .reshape(128, NDT * MCOLS))
        wkvs = np.concatenate([Wk[:, ks], Wv[:, ks]], axis=1).astype(bf)
        wkvr = np.ascontiguousarray(
            wkvs.reshape(NDT, 128, 128).transpose(1, 0, 2)
            .reshape(128, NDT * 128))
        bq_c = np.ascontiguousarray((bq[qs] * sc).reshape(2, 128).T)
        in_maps.append({
            "hsr": hsr,
            "wqr": wqr,
            "wkvr": wkvr,
            "bq": bq_c,
            "bkv": np.concatenate([bk[ks], bv[ks]]).reshape(128, 1),
            "ident": ident,
        })
    return in_maps


_NC_CACHE = {}


def get_nc():
    if "nc" not in _NC_CACHE:
        _NC_CACHE["nc"] = build_nc()
    return _NC_CACHE["nc"]


def kernel(hidden_states, Wq, bq, Wk, bk, Wv, bv):
    nc = get_nc()
    in_maps = make_in_maps(hidden_states, Wq, bq, Wk, bk, Wv, bv)
    res = run_bass_kernel_spmd(nc, in_maps, list(range(NCORES)))
    outs = [np.asarray(r["out"], dtype=np.float32) for r in res.results]
    return np.concatenate(outs, axis=-1)


# revision 28
# speedup vs baseline: 1.8998x; 1.0139x over previous
"""Trainium2 Bass kernel for GroupedQueryAttention (v3, bf16, early-start).

Sharding: 8 cores; core c owns KV head g=c and Q heads 4c..4c+3, both batch
elements. Each core computes its [2, 2048, 256] output slice; host concats.

Design:
  * hs is pre-transposed AND pre-cast to bf16 on the host (hsr layout:
    [b, s-chunk, d-in-tile(128), d-tile(16) x s(512)]), removing all 512
    PE transposes and their DVE drains from the device program.
  * All PE operands are bf16 (1.0 cycles/row at any ap size), PSUM fp32.
  * PV uses expS^T tiles as the STATIONARY and natural [V|1] as the moving
    ([128 k, 65]): output is ctx in natural [q, d] orientation with the
    softmax denominator in column 64 -> no output transposes, and PV costs
    65 rows per (k-tile, q-tile) instead of 512 per (k-tile, 512q).
  * exp runs on ACT in [128, 1024] chunks (double-buffered PSUM); ACT is
    the global bottleneck (~267us busy), so the schedule keeps it saturated:
    - early start: attention chunk (qc0,h0) k-tiles 0..7 begin right after
      hs-chunks 0,1 are projected; hs-chunks 2,3 interleave into those
      iterations, so the first exp fires ~35us earlier than a sequential
      phase-A/phase-C split;
    - batch 1's entire projection phase is interleaved into batch 0's
      attention loop (~2 PE instructions per k-tile iteration);
    - PV for chunk (qc,h) is software-pipelined into the scores/exp loop of
      the next chunk so ctx accumulation groups stay sequential per PSUM
      zero region (hardware allows one open group per 2KB bank).
"""

import sys
from contextlib import ExitStack

import numpy as np

sys.path.insert(0, "/opt/trn_rl_repo")

import concourse.bass as bass  # noqa: E402
import concourse.bacc as bacc  # noqa: E402
import concourse.tile as tile  # noqa: E402
from concourse import mybir  # noqa: E402
from concourse.bass_utils import run_bass_kernel_spmd  # noqa: E402

B = 2
S = 2048
D = 2048
HD = 64
NCORES = 8
QH = 4           # q heads per core
MCOLS = QH * HD  # 256 output cols per core

BF = mybir.dt.bfloat16
F32 = mybir.dt.float32
Exp = mybir.ActivationFunctionType.Exp

NDT = 16         # d tiles of 128
NSC = 4          # s chunks of 512 per batch
NKT = 16         # s_k tiles of 128
NQC = 2          # q chunks of 1024 per batch
QTPC = 8         # q tiles of 128 per q chunk


def build_nc():
    nc = bacc.Bacc("TRN2", target_bir_lowering=False, debug=False)

    hsr_d = nc.dram_tensor("hsr", [B, NSC, 128, NDT * 512], BF,
                           kind="ExternalInput")
    wqr_d = nc.dram_tensor("wqr", [128, NDT * 256], BF, kind="ExternalInput")
    wkvr_d = nc.dram_tensor("wkvr", [128, NDT * 128], BF, kind="ExternalInput")
    bq_d = nc.dram_tensor("bq", [128, 2], F32, kind="ExternalInput")
    bkv_d = nc.dram_tensor("bkv", [128, 1], F32, kind="ExternalInput")
    id_d = nc.dram_tensor("ident", [128, 128], BF, kind="ExternalInput")
    out_d = nc.dram_tensor("out", [B, S, MCOLS], F32, kind="ExternalOutput")

    with tile.TileContext(nc) as tc, ExitStack() as ctx:
        const = ctx.enter_context(tc.tile_pool(name="const", bufs=1))
        wqp = ctx.enter_context(tc.tile_pool(name="wqp", bufs=1))
        hstp = ctx.enter_context(tc.tile_pool(name="hstp", bufs=4))
        qtp = ctx.enter_context(tc.tile_pool(name="qtp", bufs=4))
        kvp = ctx.enter_context(tc.tile_pool(name="kvp", bufs=2))
        kthp = ctx.enter_context(tc.tile_pool(name="kthp", bufs=2))
        v1p = ctx.enter_context(tc.tile_pool(name="v1p", bufs=2 * NKT))
        expp = ctx.enter_context(tc.tile_pool(name="expp", bufs=38))
        recp = ctx.enter_context(tc.tile_pool(name="recp", bufs=4))
        outp = ctx.enter_context(tc.tile_pool(name="outp", bufs=16))
        psap = ctx.enter_context(tc.tile_pool(name="psap", bufs=2, space="PSUM"))
        pssp = ctx.enter_context(tc.tile_pool(name="pssp", bufs=2, space="PSUM"))
        ctxp = ctx.enter_context(tc.tile_pool(name="ctxp", bufs=1, space="PSUM"))

        # DMA order is the cold-start critical path: Wq first, then hs chunk
        # 0, so the first projection chain can begin ~9us in; everything else
        # follows.
        wq_sb = wqp.tile([128, NDT * 256], BF, tag="wq")
        nc.sync.dma_start(out=wq_sb[:], in_=wqr_d[:])
        hst = [[None] * NSC for _ in range(B)]
        for b in range(B):
            for sc in range(NSC):
                hst[b][sc] = hstp.tile([128, NDT * 512], BF, tag="hst",
                                       name=f"hst{b}_{sc}")
        nc.sync.dma_start(out=hst[0][0][:], in_=hsr_d[0, 0])
        wkv_sb = wqp.tile([128, NDT * 128], BF, tag="wkv")
        nc.sync.dma_start(out=wkv_sb[:], in_=wkvr_d[:])
        ident = const.tile([128, 128], BF, tag="ident")
        nc.sync.dma_start(out=ident[:], in_=id_d[:])
        bq_sb = const.tile([128, 2], F32, tag="bq")
        nc.sync.dma_start(out=bq_sb[:], in_=bq_d[:])
        bkv_sb = const.tile([128, 1], F32, tag="bkv")
        nc.sync.dma_start(out=bkv_sb[:], in_=bkv_d[:])
        zb = const.tile([128, 1], F32, tag="zb")
        nc.vector.memset(zb[:], 0.0)

        # PE p-state warmup: the tensor engine only reaches full clock after
        # ~3us of continuous work, and the first real chain otherwise pays
        # the slow ramp right when ACT is starved for scores.  Burn the
        # initial weight/hs DMA wait (~11us) on dummy matmuls over a memset
        # scratch tile so the ramp completes before real work arrives.
        scr = const.tile([128, 512], BF, tag="scr")
        nc.vector.memset(scr[:], 0.0)
        for _ in range(40):
            ps = psap.tile([128, 512], F32, tag="ps")
            nc.tensor.matmul(ps[:], scr[:, 0:128], scr[:], start=True,
                             stop=True)
        for b in range(B):
            for sc in range(NSC):
                if (b, sc) == (0, 0):
                    continue
                nc.sync.dma_start(out=hst[b][sc][:], in_=hsr_d[b, sc])

        qT = [[None, None] for _ in range(B)]
        kvT = [None] * B
        kth = [None] * B
        # V tiles pre-created with their ones column set while the DMAs of
        # the first hs chunks are still in flight (DVE is idle then).
        v1 = [[None] * NKT for _ in range(B)]
        for b in range(B):
            for kt in range(NKT):
                v = v1p.tile([128, 65], BF, tag="v1", name=f"v1_{b}_{kt}")
                nc.vector.memset(v[:, 64:65], 1.0)
                v1[b][kt] = v

        def init_b(b):
            qT[b][0] = qtp.tile([128, S], BF, tag="qt", name=f"qT{b}_0")
            qT[b][1] = qtp.tile([128, S], BF, tag="qt", name=f"qT{b}_1")
            kvT[b] = kvp.tile([128, S], BF, tag="kv", name=f"kvT{b}")
            kth[b] = kthp.tile([128, S], BF, tag="kth", name=f"kth{b}")

        def q_chain(b, sc, qc):
            hs_t = hst[b][sc]
            c0 = sc * 512
            ps = psap.tile([128, 512], F32, tag="ps")
            for dt_ in range(NDT):
                nc.tensor.matmul(
                    ps[:],
                    wq_sb[:, dt_ * 256 + qc * 128:dt_ * 256 + (qc + 1) * 128],
                    hs_t[:, dt_ * 512:(dt_ + 1) * 512],
                    start=(dt_ == 0), stop=(dt_ == NDT - 1),
                )
                yield
            nc.vector.tensor_scalar_add(
                qT[b][qc][:, c0:c0 + 512], ps[:], bq_sb[:, qc:qc + 1])
            yield

        def chunk_q0kv(b, sc):
            """Q heads 0/1 + KV projections, kth copy and V tiles for hs
            chunk sc.  Everything attention on heads 0/1 needs; the heads
            2/3 projection (q_chain qc=1) can run much later."""
            hs_t = hst[b][sc]
            c0 = sc * 512
            yield from q_chain(b, sc, 0)
            ps = psap.tile([128, 512], F32, tag="ps")
            for dt_ in range(NDT):
                nc.tensor.matmul(
                    ps[:], wkv_sb[:, dt_ * 128:(dt_ + 1) * 128],
                    hs_t[:, dt_ * 512:(dt_ + 1) * 512],
                    start=(dt_ == 0), stop=(dt_ == NDT - 1),
                )
                yield
            nc.vector.tensor_scalar_add(
                kvT[b][:, c0:c0 + 512], ps[:], bkv_sb[:])
            yield
            # K^T rows shifted to partitions 64:127 for odd heads; issued
            # from the Pool queue so the SP queue (hsT loads) is not blocked.
            nc.gpsimd.dma_start(out=kth[b][64:128, c0:c0 + 512],
                                in_=kvT[b][0:64, c0:c0 + 512])
            yield
            # V natural tiles [s_k 128, 64]; the ones column was pre-set.
            for kt in range(sc * 4, sc * 4 + 4):
                pst = psap.tile([128, 512], BF, tag="ps", name=f"pst{b}_{kt}")
                nc.tensor.transpose(
                    pst[:, 0:64], kvT[b][64:128, kt * 128:(kt + 1) * 128],
                    ident[64:128, 64:128],
                )
                yield
                nc.vector.tensor_copy(v1[b][kt][:, 0:64], pst[:, 0:64])
                yield

        def gen_a(b):
            init_b(b)
            for sc in range(NSC):
                yield from chunk_q0kv(b, sc)
                yield from q_chain(b, sc, 1)

        # ---- phase C machinery ----
        outt = {0: [None] * (NQC * QTPC), 1: [None] * (NQC * QTPC)}
        exs = {}
        ctxs = {}

        def coff(qi):
            # qi 0..6 packed in bank 0; qi 7 at the bank-1 boundary so no
            # accumulation group straddles a PSUM bank.
            return qi * 65 if qi < 7 else 512

        def ex_stat(bkey, kt, qi):
            for ex, qi0, nqi in exs[bkey][kt]:
                if qi0 <= qi < qi0 + nqi:
                    j = qi - qi0
                    return ex[:, j * 128:(j + 1) * 128]
            raise AssertionError(f"no exp span for {bkey} kt={kt} qi={qi}")

        def pv_chunk(bkey, it, swap=False):
            # it 0..15: two passes of 8 k-tiles per q-tile qi = it//2.
            # swap=True consumes k-tiles 8..15 on the first pass (used when
            # the producing chunk emitted its exps in swapped order).
            b, qc, h = bkey
            ctx_t = ctxs[bkey]
            qi = it // 2
            base = (it % 2) * 8
            if swap:
                base = 8 - base
            for k2 in range(8):
                kt = base + k2
                nc.tensor.matmul(
                    ctx_t[:, coff(qi):coff(qi) + 65],
                    ex_stat(bkey, kt, qi),
                    v1[b][kt][:],
                    start=(it % 2 == 0 and k2 == 0),
                    stop=(it % 2 == 1 and k2 == 7),
                )

        def normalize_qi(bkey, qi):
            b, qc, h = bkey
            ctx_t = ctxs[bkey]
            qtile = qc * QTPC + qi
            if h == 0:
                outt[b][qtile] = outp.tile([128, MCOLS], F32, tag="out",
                                           name=f"outt{b}_{qtile}")
            rec = recp.tile([128, 1], F32, tag="rec")
            nc.vector.reciprocal(
                rec[:], ctx_t[:, coff(qi) + 64:coff(qi) + 65])
            nc.vector.tensor_scalar_mul(
                outt[b][qtile][:, h * 64:(h + 1) * 64],
                ctx_t[:, coff(qi):coff(qi) + 64], rec[:])
            if h == QH - 1:
                nc.sync.dma_start(
                    out=out_d[b, qtile * 128:(qtile + 1) * 128, :],
                    in_=outt[b][qtile][:])

        def begin_chunk(bkey):
            b, qc, h = bkey
            ctxs[bkey] = ctxp.tile([128, 577], F32, tag="ctx",
                                   name=f"ctx{b}_{qc}_{h}")
            exs[bkey] = []

        def score_exp(bkey, kt, qi0=0, nqi=QTPC):
            b, qc, h = bkey
            r0 = (h % 2) * 64
            kmat = kvT[b] if r0 == 0 else kth[b]
            qt = qT[b][h // 2]
            w = nqi * 128
            pss = pssp.tile([128, 1024], F32, tag="pss")
            for j in range(0, w, 512):
                jw = min(512, w - j)
                q0 = qc * 1024 + qi0 * 128 + j
                nc.tensor.matmul(
                    pss[:, j:j + jw],
                    kmat[r0:r0 + 64, kt * 128:(kt + 1) * 128],
                    qt[r0:r0 + 64, q0:q0 + jw],
                    start=True, stop=True,
                )
            ex = expp.tile([128, 1024], BF, tag="ex")
            nc.scalar.activation(ex[:, 0:w], pss[:, 0:w], Exp, bias=zb[:])
            while len(exs[bkey]) <= kt:
                exs[bkey].append([])
            exs[bkey][kt].append((ex, qi0, nqi))

        def finish_chunk(bkey):
            del exs[bkey]
            del ctxs[bkey]

        def advance(gen, n):
            if gen is None:
                return None
            for _ in range(n):
                if next(gen, StopIteration) is StopIteration:
                    return None
            return gen

        # ---- emission ----
        # Batch 0 prologue: after hs chunk 0's Q0/KV projections, scores/exp
        # for (qc0, h0) start immediately on the q/k tiles that exist,
        # widening as later chunks land.  Each 4-iteration phase interleaves
        # exactly the projection work the NEXT phase needs, so batch 0's
        # bulk is spread across the whole prologue instead of serializing
        # in front of it.
        init_b(0)
        for _ in chunk_q0kv(0, 0):
            pass
        key00 = (0, 0, 0)
        begin_chunk(key00)
        phases = [
            (range(4), 0, 4, chunk_q0kv(0, 1), 11),   # k 0:512 x q 0:512
            (range(4), 4, 4, chunk_q0kv(0, 2), 11),   # k 0:512 x q 512:1024
            (range(4, 8), 0, 8, chunk_q0kv(0, 3), 11),
            (range(8, 12), 0, 8, q_chain(0, 0, 1), 5),
            (range(12, 16), 0, 8, q_chain(0, 1, 1), 5),
        ]
        for kts, qi0, nqi, g, rate in phases:
            for kt in kts:
                score_exp(key00, kt, qi0, nqi)
                g = advance(g, rate)
            if g is not None:
                for _ in g:
                    pass

        # Steady state: remaining 7 chunks of batch 0 with the deferred
        # batch-0 head-2/3 projections and batch 1's full projection phase
        # interleaved at an adaptive rate, then batch 1's chunks.
        order0 = [(0, qc, h) for qc in range(NQC) for h in range(QH)][1:]
        order1 = [(1, qc, h) for qc in range(NQC) for h in range(QH)]

        def steady_units():
            yield from q_chain(0, 2, 1)
            yield from q_chain(0, 3, 1)
            init_b(1)
            for sc in range(NSC):
                yield from chunk_q0kv(1, sc)
                if sc < 2:
                    # heads 2/3 projections for hs chunks 2,3 are deferred
                    # into batch 1's own attention loop (needed at its
                    # iteration 96, not before it starts).
                    yield from q_chain(1, sc, 1)

        g1 = steady_units()
        prev = key00
        for bkey in order0:
            begin_chunk(bkey)
            for kt in range(NKT):
                score_exp(bkey, kt)
                pv_chunk(prev, kt)
                if kt % 2 == 1:
                    normalize_qi(prev, kt // 2)
                g1 = advance(g1, 2)
            finish_chunk(prev)
            prev = bkey
        # flush batch 1 projections before its attention begins
        if g1 is not None:
            for _ in g1:
                pass
        def b1_tail():
            yield from q_chain(1, 2, 1)
            yield from q_chain(1, 3, 1)

        gt = b1_tail()
        for bkey in order1:
            last = bkey == order1[-1]
            begin_chunk(bkey)
            for i, kt in enumerate(range(NKT)):
                # The final chunk emits k-tiles 8..15 first so its PV flush
                # (which consumes the late k-tiles on odd passes) is never
                # waiting on the exp backlog at the very end.
                score_exp(bkey, (kt + 8) % NKT if last else kt)
                pv_chunk(prev, kt)
                if kt % 2 == 1:
                    normalize_qi(prev, kt // 2)
                gt = advance(gt, 2)
            finish_chunk(prev)
            prev = bkey
        for it in range(NKT):
            pv_chunk(prev, it, swap=True)
            if it % 2 == 1:
                normalize_qi(prev, it // 2)
        finish_chunk(prev)

    nc.compile()
    return nc


def make_in_maps(hidden_states, Wq, bq, Wk, bk, Wv, bv):
    bf = mybir.dt.np(BF)
    hs = np.asarray(hidden_states, dtype=np.float32)
    Wq = np.asarray(Wq, dtype=np.float32)
    bq = np.asarray(bq, dtype=np.float32)
    Wk = np.asarray(Wk, dtype=np.float32)
    bk = np.asarray(bk, dtype=np.float32)
    Wv = np.asarray(Wv, dtype=np.float32)
    bv = np.asarray(bv, dtype=np.float32)
    sc = 1.0 / np.sqrt(np.float32(HD))
    # [b, sc, p(d in tile), t(d tile), j(s in chunk)] -> [2, 4, 128, 8192]
    hsr = np.ascontiguousarray(
        hs.reshape(B, NSC, 512, NDT, 128).transpose(0, 1, 4, 3, 2)
        .reshape(B, NSC, 128, NDT * 512).astype(bf))
    ident = np.eye(128, dtype=bf)
    in_maps = []
    for c in range(NCORES):
        qs = slice(c * MCOLS, (c + 1) * MCOLS)
        ks = slice(c * HD, (c + 1) * HD)
        wqs = (Wq[:, qs] * sc).astype(bf)
        wqr = np.ascontiguousarray(
            wqs.reshape(NDT, 128, MCOLS).transpose(1, 0, 2)
            .reshape(128, NDT * MCOLS))
        wkvs = np.concatenate([Wk[:, ks], Wv[:, ks]], axis=1).astype(bf)
        wkvr = np.ascontiguousarray(
            wkvs.reshape(NDT, 128, 128).transpose(1, 0, 2)
            .reshape(128, NDT * 128))
        bq_c = np.ascontiguousarray((bq[qs] * sc).reshape(2, 128).T)
        in_maps.append({
            "hsr": hsr,
            "wqr": wqr,
            "wkvr": wkvr,
            "bq": bq_c,
            "bkv": np.concatenate([bk[ks], bv[ks]]).reshape(128, 1),
            "ident": ident,
        })
    return in_maps


_NC_CACHE = {}


def get_nc():
    if "nc" not in _NC_CACHE:
        _NC_CACHE["nc"] = build_nc()
    return _NC_CACHE["nc"]


def kernel(hidden_states, Wq, bq, Wk, bk, Wv, bv):
    nc = get_nc()
    in_maps = make_in_maps(hidden_states, Wq, bq, Wk, bk, Wv, bv)
    res = run_bass_kernel_spmd(nc, in_maps, list(range(NCORES)))
    outs = [np.asarray(r["out"], dtype=np.float32) for r in res.results]
    return np.concatenate(outs, axis=-1)
